# revision 1
# baseline (speedup 1.0000x reference)
"""Trainium2 Bass kernel for nn_EvenLayer (LDPC min-sum check-node update).

Reference semantics (B=8 batches, E=3600 edges):
    neighbor = inf_mask == 0            # (E, E)
    signs    = sign(prod(where(neighbor, x, 1), axis=-1))
    mins     = min(|x| + inf_mask, axis=-1)
    out      = signs * max(mins - bias, 0)

The mask encodes "shares a check node, excluding self" — an equivalence
relation minus the diagonal. The host verifies that structure at runtime
(values only {0, +inf}, empty diagonal, rows = leader-equality classes);
on success each edge-group (check node, size d=6) is packed into slots,
sharded over the 8 cores, and a tiny SPMD kernel computes per slot:
    loo_min  = leave-one-out min of |x| over the group  (tournament tree)
    loo_sign = sign bit of the leave-one-out product    (tournament tree)
    out      = relu(loo_min - bias) with loo_sign OR'd into the sign bit
which is bit-exact vs the reference. If verification fails, a generic
dense kernel computes the masked reductions directly from the mask data
(including the reference's product-underflow semantics for signs).
"""

import numpy as np

B, E, NCORES = 8, 3600, 8

_NC_CACHE = {}
TRACE = False
LAST_RESULT = None  # BassKernelResults of the last run (for test harness)


def _analyze(inf_mask):
    """Return leader labels if the mask is exactly an equivalence relation
    minus the diagonal with values {0, +inf}; else None."""
    m = np.asarray(inf_mask)
    if m.ndim != 2 or m.shape[0] != m.shape[1]:
        return None
    if not np.all((m == 0) | np.isposinf(m)):
        return None
    nb = m == 0
    if nb.diagonal().any():
        return None
    n = m.shape[0]
    idx = np.arange(n)
    first = np.argmax(nb, axis=1)
    has = nb.any(axis=1)
    leader = np.where(has, np.minimum(idx, first), idx)
    eq = leader[:, None] == leader[None, :]
    np.fill_diagonal(eq, False)
    if not np.array_equal(nb, eq):
        return None
    return leader


def _build_slots(leader, nbatch=B):
    """Pack groups into (NCORES, blocks, gpb, d) slot->edge index array (-1 pad)."""
    max_blocks = max(128 // nbatch, 1)
    order = np.argsort(leader, kind="stable")
    lead_sorted = leader[order]
    uniq, counts = np.unique(lead_sorted, return_counts=True)
    G = len(uniq)
    d = max(int(counts.max()), 2)
    G8 = ((G + NCORES - 1) // NCORES) * NCORES
    slot_edge = np.full((G8, d), -1, dtype=np.int64)
    col = np.concatenate([np.arange(c) for c in counts])
    row = np.repeat(np.arange(G), counts)
    slot_edge[row, col] = order
    Gc = G8 // NCORES
    gpb = (Gc + max_blocks - 1) // max_blocks   # groups per partition-block
    blocks = (Gc + gpb - 1) // gpb
    Gcp = blocks * gpb
    slot_all = slot_edge.reshape(NCORES, Gc, d)
    if Gcp != Gc:
        pad = np.full((NCORES, Gcp - Gc, d), -1, dtype=np.int64)
        slot_all = np.concatenate([slot_all, pad], axis=1)
    return slot_all.reshape(NCORES, blocks, gpb, d), d, blocks, gpb


def _build_fast_nc(P, F, gpb, d):
    """Raw-bass kernel (no TileContext — the walrus in this container rejects
    instructions carrying >2 sync waits, which Tile's tail drain emits).

    Input "xb" packs [x_slots | bias_slots] as (P, 2F); output "ys" is (P, F).
    Per slot s of each group g (slots along the innermost dim, d per group):
        A    = |x|                               (ACT, parallel with DVE)
        GP_g = prod_s x[g,s]                     (raw product; +inf pads are
                                                  sign-neutral)
        T    = GP_g * x  -> sign(T) = sign of leave-one-out product (x^2 > 0)
        M    = leave-one-out min of A via fused prefix/suffix chain:
               Wbuf[c] = (pre[c], suf[d-1-c]) pairs, one TT-min per step
        out  = (relu(M - bias)) | signbit(T)     (bitwise or; relu >= 0)
    """
    import contextlib

    import concourse.bass as bass
    from concourse import mybir

    f32 = mybir.dt.float32
    i32 = mybir.dt.int32
    AL = mybir.AluOpType
    AX = mybir.AxisListType

    nc = bass.Bass()
    xb = nc.declare_dram_parameter("xb", [P, 2 * F], f32, isOutput=False)
    ys = nc.declare_dram_parameter("ys", [P, F], f32, isOutput=True)

    with contextlib.ExitStack() as ctx:
        XB = ctx.enter_context(nc.sbuf_tensor("XB", [P, 2 * F], f32))
        A = ctx.enter_context(nc.sbuf_tensor("A", [P, F], f32))
        T = ctx.enter_context(nc.sbuf_tensor("T", [P, F], f32))
        Ti = ctx.enter_context(nc.sbuf_tensor("Ti", [P, F], i32))
        Km = ctx.enter_context(nc.sbuf_tensor("Km", [P, F], i32))
        Kp = ctx.enter_context(nc.sbuf_tensor("Kp", [P, F], i32))
        M = ctx.enter_context(nc.sbuf_tensor("M", [P, F], f32))
        Wb = ctx.enter_context(nc.sbuf_tensor("Wb", [P, gpb, max(d - 2, 1), 2], f32))
        Wp = ctx.enter_context(nc.sbuf_tensor("Wp", [P, gpb, max(d - 2, 1), 2], f32))
        R = ctx.enter_context(nc.sbuf_tensor("R", [P, F], f32))
        O = ctx.enter_context(nc.sbuf_tensor("O", [P, F], i32))

        s_in = ctx.enter_context(nc.semaphore("s_in"))
        s_dve = ctx.enter_context(nc.semaphore("s_dve"))
        s_out = ctx.enter_context(nc.semaphore("s_out"))
        s_v = ctx.enter_context(nc.semaphore("s_v"))
        block = ctx.enter_context(nc.Block())

        X = XB[:, 0:F]
        Bt = XB[:, F : 2 * F]

        @block.sync
        def _(sync):
            sync.dma_start(out=XB[:], in_=xb[:]).then_inc(s_in, 16)
            sync.wait_ge(s_dve, 1)
            sync.dma_start(out=ys[:], in_=O[:].bitcast(f32)).then_inc(s_out, 16)
            sync.wait_ge(s_out, 16)

        @block.vector
        def _(vector):
            X3 = X.rearrange("p (g d) -> p g d", d=d)
            A3 = A[:].rearrange("p (g d) -> p g d", d=d)
            M3 = M[:].rearrange("p (g d) -> p g d", d=d)
            T3 = T[:].rearrange("p (g d) -> p g d", d=d)

            # DVE self-sem chain: every op incs s_v; dependent ops wait on the
            # producer's count (same-engine RAW through SBUF needs sync).
            # A high-water mark elides waits already covered. (Attaching
            # waits to sync_info instead was tested: identical timing — the
            # sequencer pipelines wait decode behind op execution.)
            cnt = [0]
            waited = [0]

            def emit(fn, wait=None):
                if wait is None:
                    wait = cnt[0]          # default: wait for all prior DVE ops
                if wait > waited[0]:
                    vector.wait_ge(s_v, wait)
                    waited[0] = wait
                fn().then_inc(s_v, 1)
                cnt[0] += 1
                return cnt[0]              # sem value once this op completes

            def tt(out, a, b, op, wait=None):
                return emit(
                    lambda: nc.vector.tensor_tensor(out=out, in0=a, in1=b, op=op),
                    wait=wait,
                )

            def loo_chain(src_h, src3, out_h, out3, wb_h, op, first_wait):
                """Leave-one-out reduction of `op` over the d slots of each
                group. d==6/d==4 use a tournament tree (4 / 2 ops); other d
                use a fused prefix/suffix pair chain (d ops)."""
                soff = src3.offset
                pstep, gstep = src3.ap[0], src3.ap[1]
                ooff = out3.offset
                opp, opg = out3.ap[0], out3.ap[1]

                def sv(off, apdims):  # view into src
                    return bass.AP(src_h, soff + off, [pstep, gstep] + apdims)

                if d == 2:
                    emit(
                        lambda: nc.vector.tensor_copy(
                            out3, sv(1, [[-1, 2]])
                        ),
                        wait=first_wait,
                    )
                    return
                if d == 4:
                    # mp[k] = op(A[2k], A[2k+1]); out[2k+s] = op(A[2k+1-s], mp[1-k])
                    t0 = tt(wb_h[:, :, 0, :], sv(0, [[2, 2]]), sv(1, [[2, 2]]), op,
                            wait=first_wait)
                    wb4 = wb_h[:, :, :, :]
                    mp_swap_b = bass.AP(wb_h, wb4.offset + 1, [wb4.ap[0], wb4.ap[1], [-1, 2], [0, 2]])
                    tt(bass.AP(out_h, ooff, [opp, opg, [2, 2], [1, 2]]),
                       sv(1, [[2, 2], [-1, 2]]), mp_swap_b, op, wait=t0)
                    return
                if d == 6:
                    # wb flat view: 8 contiguous slots per group; use 0..5
                    wb4 = wb_h[:, :, :, :]
                    wboff = wb4.offset
                    wv = lambda off, apdims: bass.AP(wb_h, wboff + off, [wb4.ap[0], wb4.ap[1]] + apdims)
                    # L1: mp[k] = op(A[2k], A[2k+1]), k=0..2 -> wb slots 0..2
                    t0 = tt(wv(0, [[1, 3]]), sv(0, [[2, 3]]), sv(1, [[2, 3]]), op,
                            wait=first_wait)
                    # L2: c0 = op(mp1, mp2), c1 = op(mp0, mp2) -> wb slots 3,4
                    tt(wv(3, [[1, 2]]), wv(1, [[-1, 2]]), wv(2, [[0, 2]]), op, wait=t0)
                    # c2 = op(mp0, mp1) -> wb slot 5
                    t2 = tt(wv(5, [[1, 1]]), wv(0, [[1, 1]]), wv(1, [[1, 1]]), op, wait=t0)
                    # L3: out[2k+s] = op(A[2k+1-s], c[k])
                    tt(bass.AP(out_h, ooff, [opp, opg, [2, 3], [1, 2]]),
                       sv(1, [[2, 3], [-1, 2]]), wv(3, [[1, 3], [0, 2]]), op, wait=t2)
                    return

                # generic: fused prefix/suffix pair chain
                def U(k):  # src slots (k, d-1-k); step may be negative
                    return sv(k, [[d - 1 - 2 * k, 2]])

                wb4 = wb_h[:, :, :, :]
                prev_t = emit(
                    lambda: nc.vector.tensor_copy(wb_h[:, :, 0, :], U(0)),
                    wait=first_wait,
                )
                for k in range(1, d - 2):
                    prev_t = tt(wb_h[:, :, k, :], wb_h[:, :, k - 1, :], U(k), op, wait=prev_t)
                # final chain step writes out[d-1] (pre[d-2]) and out[0] (suf[1])
                ends = bass.AP(out_h, ooff + d - 1, [opp, opg, [-(d - 1), 2]])
                tt(ends, wb_h[:, :, d - 3, :], U(d - 2), op, wait=prev_t)
                # middles: out[j] = pre[j-1] `op` suf[j+1], j = 1..d-2, one op
                pre_view = bass.AP(wb_h, wb4.offset, [wb4.ap[0], wb4.ap[1], [2, d - 2]])
                suf_rev = bass.AP(wb_h, wb4.offset + (d - 3) * 2 + 1, [wb4.ap[0], wb4.ap[1], [-2, d - 2]])
                tt(out3[:, :, 1 : d - 1], pre_view, suf_rev, op)

            # mask tiles (no data deps; run during the input DMA)
            emit(lambda: nc.vector.memset(Km[:], -2147483648), wait=0)
            t_msets = emit(lambda: nc.vector.memset(Kp[:], 2147483647), wait=0)

            vector.wait_ge(s_in, 16)
            # ---- |x| as one int32 AND (bit-exact abs, no scalar engine:
            # the first ACT activation pays a ~1.6us cold-table load) ----
            t_abs = emit(
                lambda: nc.vector.tensor_tensor(
                    out=A[:].bitcast(i32), in0=X.bitcast(i32), in1=Kp[:], op=AL.bitwise_and
                ),
                wait=t_msets,
            )
            # ---- leave-one-out product of raw x -> its sign bit (+inf pads
            # are positive, hence sign-neutral); interleaved with the min tree
            # so the wait high-water-mark elides the product tree's waits ----
            loo_chain(XB, X3, T, T3, Wp, AL.mult, first_wait=0)
            t_prod = cnt[0]
            loo_chain(A, A3, M, M3, Wb, AL.min, first_wait=t_abs)
            t_min = cnt[0]

            # ---- out = relu(M - bias) with the sign bit OR'd in ----
            t_sub = emit(lambda: nc.vector.tensor_sub(R[:], M[:], Bt), wait=t_min)
            emit(lambda: nc.vector.tensor_tensor(out=Ti[:], in0=T[:].bitcast(i32), in1=Km[:], op=AL.bitwise_and), wait=t_prod)
            emit(lambda: nc.vector.tensor_relu(out=R[:], in_=R[:]), wait=t_sub)
            vector.wait_ge(s_v, cnt[0])
            nc.vector.tensor_tensor(
                out=O[:], in0=R[:].bitcast(i32), in1=Ti[:], op=AL.bitwise_or
            ).then_inc(s_dve, 1)

    return nc


def _run_spmd(nc, in_maps):
    global LAST_RESULT
    from concourse.bass_utils import run_bass_kernel_spmd

    res = run_bass_kernel_spmd(
        nc, in_maps, core_ids=list(range(NCORES)), trace=TRACE
    )
    LAST_RESULT = res
    return res.results


def _kernel_fast(x, bias, leader):
    Bn, E_ = x.shape
    slot_all, d, blocks, gpb = _build_slots(leader, nbatch=Bn)
    P, F = Bn * blocks, gpb * d
    key = ("fast", P, F, gpb, d)
    if key not in _NC_CACHE:
        _NC_CACHE[key] = _build_fast_nc(P, F, gpb, d)
    nc = _NC_CACHE[key]

    in_maps = []
    for c in range(NCORES):
        e = slot_all[c]                       # (blocks, gpb, d)
        valid = e >= 0
        ec = np.clip(e, 0, None)
        xs = np.where(valid[None], x[:, ec], np.float32(np.inf))
        bsv = np.where(valid, bias[0, ec], np.float32(0.0))
        bsv = np.broadcast_to(bsv[None], (Bn,) + bsv.shape)
        xb = np.concatenate(
            [xs.reshape(P, F), bsv.reshape(P, F)], axis=1
        )
        in_maps.append({"xb": np.ascontiguousarray(xb, np.float32)})

    results = _run_spmd(nc, in_maps)

    out = np.empty((Bn, E_), np.float32)
    for c in range(NCORES):
        e = slot_all[c]
        valid = e >= 0
        ys = results[c]["ys"].reshape(Bn, blocks, gpb, d)
        out[:, e[valid]] = ys[:, valid]
    return out


def kernel(inputs, bias, inf_mask):
    x = np.ascontiguousarray(np.asarray(inputs), np.float32)
    bias = np.ascontiguousarray(np.asarray(bias), np.float32)
    inf_mask = np.asarray(inf_mask)

    leader = _analyze(inf_mask)
    if leader is not None:
        return _kernel_fast(x, bias, leader)
    return _kernel_dense(x, bias, inf_mask)


def _build_dense_nc(Bn, E, Ec):
    """Generic dense fallback: any (E, E) float mask, mask rows sharded
    per core (Ec rows, padded with +inf). Exactly follows the reference:
        nb    = mask == 0
        w     = nb ? x : 1       -> signs = sign(prod w)  (pairwise tree)
        mins  = min(|x| + mask)  (fused add+min reduce)
        out   = signs * max(mins - bias_row, 0)
    Output layout "ys" is (Ec, Bn) (row-major per output row; host transposes).
    """
    import contextlib

    import concourse.bass as bass
    from concourse import mybir

    f32 = mybir.dt.float32
    AL = mybir.AluOpType
    AX = mybir.AxisListType

    PT = 128
    ntiles = (Ec + PT - 1) // PT
    assert Ec % ntiles == 0 and (Ec // ntiles) <= PT
    TR = Ec // ntiles  # rows per tile

    nc = bass.Bass()
    mrows = nc.declare_dram_parameter("mrows", [Ec, E], f32, isOutput=False)
    xfull = nc.declare_dram_parameter("xfull", [Bn, E], f32, isOutput=False)
    brows = nc.declare_dram_parameter("brows", [Ec, 1], f32, isOutput=False)
    ys = nc.declare_dram_parameter("ys", [Ec, Bn], f32, isOutput=True)

    with contextlib.ExitStack() as ctx:
        XB = []
        for b in range(Bn):
            XB.append(ctx.enter_context(nc.sbuf_tensor(f"XBc{b}", [TR, E], f32)))
        MT = ctx.enter_context(nc.sbuf_tensor("MT", [TR, E], f32))
        W = ctx.enter_context(nc.sbuf_tensor("W", [TR, E], f32))
        SC = ctx.enter_context(nc.sbuf_tensor("SC", [TR, E], f32))
        SC2 = ctx.enter_context(nc.sbuf_tensor("SC2", [TR, E], f32))
        BC = ctx.enter_context(nc.sbuf_tensor("BC", [TR, 1], f32))
        MI = ctx.enter_context(nc.sbuf_tensor("MI", [TR, 1], f32))
        SG = ctx.enter_context(nc.sbuf_tensor("SG", [TR, 1], f32))
        PR = ctx.enter_context(nc.sbuf_tensor("PR", [TR, 1], f32))
        OT = ctx.enter_context(nc.sbuf_tensor("OT", [TR, Bn], f32))

        s_bc = ctx.enter_context(nc.semaphore("s_bc"))
        s_m = ctx.enter_context(nc.semaphore("s_m"))
        s_v = ctx.enter_context(nc.semaphore("s_v"))
        s_t = ctx.enter_context(nc.semaphore("s_t"))
        s_out = ctx.enter_context(nc.semaphore("s_out"))
        block = ctx.enter_context(nc.Block())

        @block.sync
        def _(sync):
            # broadcast each batch row of x across TR partitions (stride-0 AP)
            for b in range(Bn):
                src = bass.AP(xfull, b * E, [[0, TR], [1, E]])
                sync.dma_start(out=XB[b][:], in_=src).then_inc(s_bc, 16)
            for t in range(ntiles):
                if t:
                    # DVE done with tile t-1: MT/BC free, OT[t-1] complete
                    sync.wait_ge(s_t, t)
                    sync.dma_start(
                        out=ys[(t - 1) * TR : t * TR, :], in_=OT[:]
                    ).then_inc(s_out, 16)
                sync.dma_start(out=MT[:], in_=mrows[t * TR : (t + 1) * TR, :]).then_inc(s_m, 16)
                sync.dma_start(out=BC[:], in_=brows[t * TR : (t + 1) * TR, :]).then_inc(s_m, 16)
            sync.wait_ge(s_t, ntiles)
            sync.dma_start(
                out=ys[(ntiles - 1) * TR : ntiles * TR, :], in_=OT[:]
            ).then_inc(s_out, 16)
            sync.wait_ge(s_out, 16 * ntiles)

        @block.vector
        def _(vector):
            cnt = [0]
            waited = [0]

            def emit(fn, wait=None):
                if wait is None:
                    wait = cnt[0]
                if wait > waited[0]:
                    vector.wait_ge(s_v, wait)
                    waited[0] = wait
                fn().then_inc(s_v, 1)
                cnt[0] += 1
                return cnt[0]

            vector.wait_ge(s_bc, 16 * Bn)
            for t in range(ntiles):
                vector.wait_ge(s_m, 32 * (t + 1))
                if t:
                    # OT(t-1) out-DMA must have completed before rewriting OT
                    vector.wait_ge(s_out, 16 * t)
                # neighbor indicator for this tile's mask rows
                emit(lambda: nc.vector.tensor_single_scalar(out=W[:], in_=MT[:], scalar=0.0, op=AL.is_equal))
                for b in range(Bn):
                    # |x| for this batch into SC2
                    emit(lambda b=b: nc.vector.tensor_scalar_mul(SC2[:], XB[b][:], -1.0))
                    emit(lambda b=b: nc.vector.tensor_max(SC2[:], SC2[:], XB[b][:]))
                    # mins = reduce-min(mask + |x|)
                    emit(lambda: nc.vector.tensor_add(SC[:], MT[:], SC2[:]))
                    emit(lambda: nc.vector.tensor_reduce(
                        out=MI[:], in_=SC[:], axis=AX.X, op=AL.min))
                    # w = W * (x - 1) + 1  (= x where nb, else 1)
                    emit(lambda b=b: nc.vector.tensor_scalar_add(SC[:], XB[b][:], -1.0))
                    emit(lambda: nc.vector.tensor_mul(SC[:], W[:], SC[:]))
                    emit(lambda: nc.vector.tensor_scalar_add(SC[:], SC[:], 1.0))
                    # signs via pairwise product tree (reproduces fp underflow)
                    n = E
                    cur, other = SC, SC2
                    while n > 1:
                        h = n // 2
                        ce = cur[:, 0 : 2 * h].rearrange("p (h two) -> p h two", two=2)
                        emit(lambda ce=ce, other=other, h=h: nc.vector.tensor_tensor(
                            out=other[:, 0:h], in0=ce[:, :, 0:1], in1=ce[:, :, 1:2], op=AL.mult))
                        if n % 2:
                            emit(lambda cur=cur, other=other, n=n: nc.vector.tensor_mul(
                                other[:, 0:1], other[:, 0:1], cur[:, n - 1 : n]))
                        cur, other = other, cur
                        n = h
                    # SG = sign(prod) = is_gt - is_lt
                    emit(lambda cur=cur: nc.vector.tensor_single_scalar(out=SG[:], in_=cur[:, 0:1], scalar=0.0, op=AL.is_gt))
                    emit(lambda cur=cur: nc.vector.tensor_single_scalar(out=PR[:], in_=cur[:, 0:1], scalar=0.0, op=AL.is_lt))
                    emit(lambda: nc.vector.tensor_sub(SG[:], SG[:], PR[:]))
                    # out col = SG * max(mins - bias, 0)
                    emit(lambda: nc.vector.tensor_scalar(
                        out=MI[:], in0=MI[:], scalar1=BC[:], scalar2=0.0,
                        op0=AL.subtract, op1=AL.max))
                    emit(lambda b=b: nc.vector.tensor_mul(OT[:, b : b + 1], SG[:], MI[:]))
                vector.wait_ge(s_v, cnt[0])
                nc.vector.engine_nop().then_inc(s_t, 1)

    return nc


def _kernel_dense(x, bias, inf_mask):
    Bn, E = x.shape
    m = np.ascontiguousarray(np.asarray(inf_mask), np.float32)
    Ec = -(-E // NCORES)
    # round Ec up so it splits into <=128-row tiles evenly
    PT = 128
    ntiles = -(-Ec // PT)
    Ec = ntiles * PT if Ec > PT else Ec
    key = ("dense", Bn, E, Ec)
    if key not in _NC_CACHE:
        _NC_CACHE[key] = _build_dense_nc(Bn, E, Ec)
    nc = _NC_CACHE[key]

    in_maps = []
    for c in range(NCORES):
        lo = c * Ec
        rows = np.full((Ec, E), np.float32(np.inf), np.float32)
        bcol = np.zeros((Ec, 1), np.float32)
        hi = min(lo + Ec, E)
        if hi > lo:
            rows[: hi - lo] = m[lo:hi]
            bcol[: hi - lo, 0] = bias[0, lo:hi]
        in_maps.append(
            {
                "mrows": rows,
                "xfull": np.ascontiguousarray(x, np.float32),
                "brows": bcol,
            }
        )

    results = _run_spmd(nc, in_maps)

    out = np.empty((Bn, E), np.float32)
    for c in range(NCORES):
        lo = c * Ec
        hi = min(lo + Ec, E)
        if hi > lo:
            out[:, lo:hi] = results[c]["ys"][: hi - lo].T
    return out



# revision 4
# speedup vs baseline: 7.2406x; 7.2406x over previous
"""Trainium2 Bass kernel for nn_EvenLayer (LDPC min-sum check-node update).

Reference semantics (B=8 batches, E=3600 edges):
    neighbor = inf_mask == 0            # (E, E)
    signs    = sign(prod(where(neighbor, x, 1), axis=-1))
    mins     = min(|x| + inf_mask, axis=-1)
    out      = signs * max(mins - bias, 0)

The mask encodes "shares a check node, excluding self" — an equivalence
relation minus the diagonal. The host verifies that structure at runtime
(values only {0, +inf}, empty diagonal, rows = leader-equality classes);
on success each edge-group (check node, size d=6) is packed into slots,
sharded over the 8 cores, and a small SPMD kernel computes per slot:
    loo_min  = leave-one-out min of |x| over the group  (tournament tree)
    loo_sign = sign bit of the leave-one-out product    (tournament tree)
    out      = relu(loo_min - bias) with loo_sign OR'd into the sign bit
which is bit-exact vs the reference.

Data movement uses the GPSIMD (Pool-engine) indirect-DMA path:
    in : dma_gather   (DRAM row i -> SBUF partition i, identity indices)
    out: dma_scatter_add (SBUF partition i -> DRAM row i; the runtime
         pre-zeros ExternalOutput buffers, so the add is a plain write)
with the index vector generated on-device via iota. All compute runs on
the Pool engine; the whole program is single-engine with no heavyweight
HWDGE legs on the critical path.

If mask verification fails, a generic dense kernel computes the masked
reductions directly from the mask data (including the reference's
product-underflow semantics for signs).
"""

import numpy as np

B, E, NCORES = 8, 3600, 8
RPAD = 128          # gather/scatter partition count (fixed by the ISA)
IDXC = RPAD // 16   # idx columns (idxs wrapped in 16 partitions)
SRC_ROWS = 256      # DRAM rows; idx tile garbage partitions reach 127+16*7=239

_NC_CACHE = {}
TRACE = False
LAST_RESULT = None  # BassKernelResults of the last run (for test harness)


def _analyze(inf_mask):
    """Return leader labels if the mask is exactly an equivalence relation
    minus the diagonal with values {0, +inf}; else None."""
    m = np.asarray(inf_mask)
    if m.ndim != 2 or m.shape[0] != m.shape[1]:
        return None
    if not np.all((m == 0) | np.isposinf(m)):
        return None
    nb = m == 0
    if nb.diagonal().any():
        return None
    n = m.shape[0]
    idx = np.arange(n)
    first = np.argmax(nb, axis=1)
    has = nb.any(axis=1)
    leader = np.where(has, np.minimum(idx, first), idx)
    eq = leader[:, None] == leader[None, :]
    np.fill_diagonal(eq, False)
    if not np.array_equal(nb, eq):
        return None
    return leader


def _build_slots(leader, nbatch=B):
    """Pack groups into (NCORES, blocks, gpb, d) slot->edge index array (-1 pad)."""
    max_blocks = max(128 // nbatch, 1)
    order = np.argsort(leader, kind="stable")
    lead_sorted = leader[order]
    uniq, counts = np.unique(lead_sorted, return_counts=True)
    G = len(uniq)
    d = max(int(counts.max()), 2)
    G8 = ((G + NCORES - 1) // NCORES) * NCORES
    slot_edge = np.full((G8, d), -1, dtype=np.int64)
    col = np.concatenate([np.arange(c) for c in counts])
    row = np.repeat(np.arange(G), counts)
    slot_edge[row, col] = order
    Gc = G8 // NCORES
    gpb = (Gc + max_blocks - 1) // max_blocks   # groups per partition-block
    blocks = (Gc + gpb - 1) // gpb
    Gcp = blocks * gpb
    slot_all = slot_edge.reshape(NCORES, Gc, d)
    if Gcp != Gc:
        pad = np.full((NCORES, Gcp - Gc, d), -1, dtype=np.int64)
        slot_all = np.concatenate([slot_all, pad], axis=1)
    return slot_all.reshape(NCORES, blocks, gpb, d), d, blocks, gpb


def _build_gather_nc(gpb, d, ew):
    """Single-engine (Pool/GPSIMD) kernel: indirect-DMA in, float-only
    compute, indirect-DMA out. Built with Bacc so GPSIMD library reloads
    are inserted and lowered automatically.

    Walrus constraints honored: Pool tensor-tensor supports only
    add/subtract/mult (f32); scalar-form min/max/is_ge are legal; no int
    alu/bitwise ops. Hence:
      sign:  S = 2*(x >= 0) - 1            (exact +/-1; +1 for +inf pads)
      abs:   A = x * S
      T:     loo-product of S (tournament) (exact +/-1)
      M:     loo-min of A (tournament; min(a,b) = b + min(a-b, 0))
      out:   max(M - bias, 0) * T

    The idx tile must hold IDX[p, c] = (p % 16) + 16*c REPLICATED in every
    16-partition stripe (the gather/scatter ucode cores read their own
    stripe; CoreSim reads stripe 0). p % 16 is built in f32 via the
    1.5*2^23 round-to-nearest trick, then copy-cast to int16.

    DRAM "xb"/"ys" are (SRC_ROWS, ew); row r < rows packs
    [x slots | bias slots | zero pad]; ys gets [out slots] per row via
    scatter-add (the runtime zero-fills ExternalOutput buffers, so add ==
    write). elem_step=ew keeps the scatter row stride 256B-aligned while
    writing only F elements per row.
    """
    import contextlib

    import concourse.bass as bass
    from concourse.bacc import Bacc
    from concourse import mybir

    f32 = mybir.dt.float32
    i16 = mybir.dt.int16
    AL = mybir.AluOpType
    F = gpb * d

    nc = Bacc(None, target_bir_lowering=False)
    xb = nc.declare_dram_parameter("xb", [SRC_ROWS, ew], f32, isOutput=False)
    ys = nc.declare_dram_parameter("ys", [SRC_ROWS, ew], f32, isOutput=True)

    with contextlib.ExitStack() as ctx:
        IDX = ctx.enter_context(nc.sbuf_tensor("IDX", [RPAD, IDXC], i16))
        PF = ctx.enter_context(nc.sbuf_tensor("PF", [RPAD, IDXC], f32))
        CF = ctx.enter_context(nc.sbuf_tensor("CF", [RPAD, IDXC], f32))
        QF = ctx.enter_context(nc.sbuf_tensor("QF", [RPAD, IDXC], f32))
        XB = ctx.enter_context(nc.sbuf_tensor("XB", [RPAD, ew], f32))
        S = ctx.enter_context(nc.sbuf_tensor("S", [RPAD, F], f32))
        A = ctx.enter_context(nc.sbuf_tensor("A", [RPAD, F], f32))
        T = ctx.enter_context(nc.sbuf_tensor("T", [RPAD, F], f32))
        M = ctx.enter_context(nc.sbuf_tensor("M", [RPAD, F], f32))
        Wp = ctx.enter_context(nc.sbuf_tensor("Wp", [RPAD, gpb, max(d - 2, 1), 2], f32))
        Wb = ctx.enter_context(nc.sbuf_tensor("Wb", [RPAD, gpb, max(d - 2, 1), 2], f32))
        SC0 = ctx.enter_context(nc.sbuf_tensor("SC0", [RPAD, gpb, 16], f32))
        SC1 = ctx.enter_context(nc.sbuf_tensor("SC1", [RPAD, gpb, 16], f32))
        SC2 = ctx.enter_context(nc.sbuf_tensor("SC2", [RPAD, gpb, 16], f32))
        SC3 = ctx.enter_context(nc.sbuf_tensor("SC3", [RPAD, gpb, 16], f32))
        R = ctx.enter_context(nc.sbuf_tensor("R", [RPAD, F], f32))
        O = ctx.enter_context(nc.sbuf_tensor("O", [RPAD, F], f32))

        s_g = ctx.enter_context(nc.semaphore("s_g"))
        s_o = ctx.enter_context(nc.semaphore("s_o"))
        s_v = ctx.enter_context(nc.semaphore("s_v"))

        X = XB[:, 0:F]
        Bt = XB[:, F : 2 * F]

        gp = nc.gpsimd
        g = gp

        cnt = [0]
        waited = [0]

        def emit(fn, wait=None):
            if wait is None:
                wait = cnt[0]          # default: wait for all prior ops
            if wait > waited[0]:
                g.wait_ge(s_v, wait)
                waited[0] = wait
            fn().then_inc(s_v, 1)
            cnt[0] += 1
            return cnt[0]

        def tt(out, a, b, op, wait=None):
            return emit(lambda: gp.tensor_tensor(out=out, in0=a, in1=b, op=op),
                        wait=wait)

        # ---- replicated idx tile (see docstring) ----
        t_p = emit(lambda: gp.iota(PF[:], pattern=[[0, IDXC]], base=0,
                                   channel_multiplier=1,
                                   allow_small_or_imprecise_dtypes=True), wait=0)
        t_c = emit(lambda: gp.iota(CF[:], pattern=[[16, IDXC]], base=0,
                                   channel_multiplier=0,
                                   allow_small_or_imprecise_dtypes=True), wait=0)
        t_q = emit(lambda: gp.tensor_scalar(
            out=QF[:], in0=PF[:], scalar1=0.0625, scalar2=0.46875,
            op0=AL.mult, op1=AL.subtract), wait=t_p)
        t_q2 = emit(lambda: gp.tensor_scalar(
            out=QF[:], in0=QF[:], scalar1=12582912.0, scalar2=12582912.0,
            op0=AL.add, op1=AL.subtract), wait=t_q)
        t_q3 = emit(lambda: gp.tensor_scalar(
            out=QF[:], in0=QF[:], scalar1=-16.0, scalar2=0.0,
            op0=AL.mult, op1=AL.add), wait=t_q2)
        t_pm = tt(PF[:], PF[:], QF[:], AL.add, wait=t_q3)
        t_ix = tt(PF[:], PF[:], CF[:], AL.add, wait=max(t_pm, t_c))
        t_setup = emit(lambda: gp.tensor_copy(IDX[:], PF[:]), wait=t_ix)

        g.wait_ge(s_v, t_setup)
        waited[0] = t_setup
        XB3 = bass.AP(XB, XB[:].offset, [XB[:].ap[0], [ew, 1], [1, ew]])
        gp.dma_gather(
            out_ap=XB3, in_ap=xb[:, :], idxs_ap=IDX[:],
            num_idxs=RPAD, num_idxs_reg=RPAD, elem_size=ew,
        ).then_inc(s_g, 16)
        g.wait_ge(s_g, 16)

        S3 = S[:].rearrange("p (g d) -> p g d", d=d)
        A3 = A[:].rearrange("p (g d) -> p g d", d=d)
        M3 = M[:].rearrange("p (g d) -> p g d", d=d)
        T3 = T[:].rearrange("p (g d) -> p g d", d=d)

        def views(src_h, src3):
            soff = src3.offset
            pstep, gstep = src3.ap[0], src3.ap[1]

            def sv(off, apdims):
                return bass.AP(src_h, soff + off, [pstep, gstep] + apdims)

            return sv

        def wviews(wb_h):
            wb4 = wb_h[:, :, :, :]
            wboff = wb4.offset

            def wv(off, apdims):
                return bass.AP(wb_h, wboff + off, [wb4.ap[0], wb4.ap[1]] + apdims)

            return wv

        def prod_tree(src_h, src3, out_h, out3, wb_h, first_wait):
            """d==6 leave-one-out product tournament (mult only)."""
            sv = views(src_h, src3)
            wv = wviews(wb_h)
            ooff = out3.offset
            opp, opg = out3.ap[0], out3.ap[1]
            op = AL.mult
            t0 = tt(wv(0, [[1, 3]]), sv(0, [[2, 3]]), sv(1, [[2, 3]]), op,
                    wait=first_wait)
            tt(wv(3, [[1, 2]]), wv(1, [[-1, 2]]), wv(2, [[0, 2]]), op, wait=t0)
            t2 = tt(wv(5, [[1, 1]]), wv(0, [[1, 1]]), wv(1, [[1, 1]]), op, wait=t0)
            tt(bass.AP(out_h, ooff, [opp, opg, [2, 3], [1, 2]]),
               sv(1, [[2, 3], [-1, 2]]), wv(3, [[1, 3], [0, 2]]), op, wait=t2)

        def min_pair(out, a, b, scr, wait):
            """out = min(a, b), bit-exact via 0/1 masks:
            d = a-b; g = [d >= 0]; h = [d < 0]; out = a*h + b*g.
            Each product multiplies by exactly 0.0 or 1.0 and one addend is
            zero, so the selected value passes through unrounded (needed:
            outputs near zero are graded at ~1e-6 absolute scale, so the
            rounding of cheaper min decompositions fails the rel-err gate)."""
            d, gm, hm, p = scr
            t0 = emit(lambda: gp.tensor_tensor(out=d, in0=a, in1=b,
                                               op=AL.subtract), wait=wait)
            t1 = emit(lambda: gp.tensor_single_scalar(out=gm, in_=d, scalar=0.0,
                                                      op=AL.is_ge), wait=t0)
            t2 = emit(lambda: gp.tensor_single_scalar(out=hm, in_=d, scalar=0.0,
                                                      op=AL.is_lt), wait=t0)
            t3 = tt(p, a, hm, AL.mult, wait=t2)
            t4 = tt(d, b, gm, AL.mult, wait=max(t1, t3))
            return tt(out, p, d, AL.add, wait=t4)

        def min_tree(src_h, src3, out_h, out3, wb_h, scr_hs, first_wait):
            """d==6 leave-one-out min tournament (exact pairwise mins)."""
            sv = views(src_h, src3)
            wv = wviews(wb_h)

            def w3views(h):
                w3 = h[:, :, :]
                w3off = w3.offset

                def wv3(off, apdims):
                    return bass.AP(h, w3off + off, [w3.ap[0], w3.ap[1]] + apdims)

                return wv3

            svs = [w3views(h) for h in scr_hs]
            ooff = out3.offset
            opp, opg = out3.ap[0], out3.ap[1]

            def scr(off, dims):
                return tuple(v(off, dims) for v in svs)

            t0 = min_pair(wv(0, [[1, 3]]), sv(0, [[2, 3]]), sv(1, [[2, 3]]),
                          scr(0, [[1, 3]]), wait=first_wait)
            min_pair(wv(3, [[1, 2]]), wv(1, [[-1, 2]]), wv(2, [[0, 2]]),
                     scr(3, [[1, 2]]), wait=t0)
            t2 = min_pair(wv(5, [[1, 1]]), wv(0, [[1, 1]]), wv(1, [[1, 1]]),
                          scr(5, [[1, 1]]), wait=t0)
            min_pair(bass.AP(out_h, ooff, [opp, opg, [2, 3], [1, 2]]),
                     sv(1, [[2, 3], [-1, 2]]), wv(3, [[1, 3], [0, 2]]),
                     tuple(v(8, [[1, 6]]) for v in svs),
                     wait=t2)

        t_s01 = emit(lambda: gp.tensor_single_scalar(
            out=S[:], in_=X, scalar=0.0, op=AL.is_ge))
        t_sgn = emit(lambda: gp.tensor_scalar(
            out=S[:], in0=S[:], scalar1=2.0, scalar2=1.0,
            op0=AL.mult, op1=AL.subtract), wait=t_s01)
        t_abs = tt(A[:], X, S[:], AL.mult, wait=t_sgn)
        prod_tree(S, S3, T, T3, Wp, first_wait=t_sgn)
        t_prod = cnt[0]
        min_tree(A, A3, M, M3, Wb, [SC0, SC1, SC2, SC3], first_wait=t_abs)
        t_min = cnt[0]
        t_sub = emit(lambda: gp.tensor_sub(R[:], M[:], Bt), wait=t_min)
        t_relu = emit(lambda: gp.tensor_scalar_max(R[:], R[:], 0.0), wait=t_sub)
        t_o = tt(O[:], R[:], T[:], AL.mult, wait=max(t_relu, t_prod))

        g.wait_ge(s_v, t_o)
        O3 = bass.AP(O, O[:].offset, [O[:].ap[0], [F, 1], [1, F]])
        ys_ap = bass.AP(ys, 0, [[ew, SRC_ROWS], [1, F]])
        gp.dma_scatter_add(
            out_ap=ys_ap, in_ap=O3, idxs_ap=IDX[:],
            num_idxs=RPAD, num_idxs_reg=RPAD, elem_size=F, elem_step=ew,
        ).then_inc(s_o, 16)
        g.wait_ge(s_o, 16)

    nc.finalize()
    return nc


def _prepare_gather(x, bias, leader):
    """Build (nc, in_maps, unpack) for the gather-kernel path, or None if the
    problem shape doesn't fit it."""
    Bn, E_ = x.shape
    slot_all, d, blocks, gpb = _build_slots(leader, nbatch=Bn)
    rows = Bn * blocks
    F = gpb * d
    ew = ((2 * F + 63) // 64) * 64      # gather/scatter element: 256B aligned
    if d != 6 or rows > RPAD or ew > SRC_ROWS:
        return None

    key = ("gather", gpb, d, ew)
    if key not in _NC_CACHE:
        _NC_CACHE[key] = _build_gather_nc(gpb, d, ew)
    nc = _NC_CACHE[key]

    in_maps = []
    for c in range(NCORES):
        e = slot_all[c]                       # (blocks, gpb, d)
        valid = e >= 0
        ec = np.clip(e, 0, None)
        xs = np.where(valid[None], x[:, ec], np.float32(1e30))
        bsv = np.where(valid, bias[0, ec], np.float32(0.0))
        bsv = np.broadcast_to(bsv[None], (Bn,) + bsv.shape)
        src = np.zeros((SRC_ROWS, ew), np.float32)
        src[:rows, 0:F] = xs.reshape(rows, F)
        src[:rows, F : 2 * F] = bsv.reshape(rows, F)
        in_maps.append({"xb": src})

    def unpack(results):
        out = np.empty((Bn, E_), np.float32)
        for c in range(NCORES):
            e = slot_all[c]
            valid = e >= 0
            ysr = results[c]["ys"][:rows, 0:F].reshape(Bn, blocks, gpb, d)
            out[:, e[valid]] = ysr[:, valid]
        return out

    return nc, in_maps, unpack


def _run_spmd(nc, in_maps):
    global LAST_RESULT
    from concourse.bass_utils import run_bass_kernel_spmd

    res = run_bass_kernel_spmd(
        nc, in_maps, core_ids=list(range(NCORES)), trace=TRACE
    )
    LAST_RESULT = res
    return res.results


def kernel(inputs, bias, inf_mask):
    x = np.ascontiguousarray(np.asarray(inputs), np.float32)
    bias = np.ascontiguousarray(np.asarray(bias), np.float32)
    inf_mask = np.asarray(inf_mask)

    leader = _analyze(inf_mask)
    if leader is not None:
        prep = _prepare_gather(x, bias, leader)
        if prep is not None:
            nc, in_maps, unpack = prep
            return unpack(_run_spmd(nc, in_maps))
        return _kernel_fast(x, bias, leader)
    return _kernel_dense(x, bias, inf_mask)


# ---------------------------------------------------------------------------
# Fallback 1: HWDGE-DMA + DVE kernel for equivalence-class masks whose shape
# doesn't fit the gather kernel (d != 6 or too many rows).
# ---------------------------------------------------------------------------


def _build_fast_nc(P, F, gpb, d):
    """Raw-bass kernel: one DMA in, DVE compute, one DMA out.

    Input "xb" packs [x_slots | bias_slots] as (P, 2F); output "ys" is (P, F).
    """
    import contextlib

    import concourse.bass as bass
    from concourse import mybir

    f32 = mybir.dt.float32
    i32 = mybir.dt.int32
    AL = mybir.AluOpType

    nc = bass.Bass()
    xb = nc.declare_dram_parameter("xb", [P, 2 * F], f32, isOutput=False)
    ys = nc.declare_dram_parameter("ys", [P, F], f32, isOutput=True)

    with contextlib.ExitStack() as ctx:
        XB = ctx.enter_context(nc.sbuf_tensor("XB", [P, 2 * F], f32))
        A = ctx.enter_context(nc.sbuf_tensor("A", [P, F], f32))
        T = ctx.enter_context(nc.sbuf_tensor("T", [P, F], f32))
        Ti = ctx.enter_context(nc.sbuf_tensor("Ti", [P, F], i32))
        Km = ctx.enter_context(nc.sbuf_tensor("Km", [P, F], i32))
        Kp = ctx.enter_context(nc.sbuf_tensor("Kp", [P, F], i32))
        M = ctx.enter_context(nc.sbuf_tensor("M", [P, F], f32))
        Wb = ctx.enter_context(nc.sbuf_tensor("Wb", [P, gpb, max(d - 2, 1), 2], f32))
        Wp = ctx.enter_context(nc.sbuf_tensor("Wp", [P, gpb, max(d - 2, 1), 2], f32))
        R = ctx.enter_context(nc.sbuf_tensor("R", [P, F], f32))
        O = ctx.enter_context(nc.sbuf_tensor("O", [P, F], i32))

        s_in = ctx.enter_context(nc.semaphore("s_in"))
        s_dve = ctx.enter_context(nc.semaphore("s_dve"))
        s_out = ctx.enter_context(nc.semaphore("s_out"))
        s_v = ctx.enter_context(nc.semaphore("s_v"))
        block = ctx.enter_context(nc.Block())

        X = XB[:, 0:F]
        Bt = XB[:, F : 2 * F]

        @block.sync
        def _(sync):
            sync.dma_start(out=XB[:], in_=xb[:]).then_inc(s_in, 16)
            sync.wait_ge(s_dve, 1)
            sync.dma_start(out=ys[:], in_=O[:].bitcast(f32)).then_inc(s_out, 16)
            sync.wait_ge(s_out, 16)

        @block.vector
        def _(vector):
            X3 = X.rearrange("p (g d) -> p g d", d=d)
            A3 = A[:].rearrange("p (g d) -> p g d", d=d)
            M3 = M[:].rearrange("p (g d) -> p g d", d=d)
            T3 = T[:].rearrange("p (g d) -> p g d", d=d)

            cnt = [0]
            waited = [0]

            def emit(fn, wait=None):
                if wait is None:
                    wait = cnt[0]
                if wait > waited[0]:
                    vector.wait_ge(s_v, wait)
                    waited[0] = wait
                fn().then_inc(s_v, 1)
                cnt[0] += 1
                return cnt[0]

            def tt(out, a, b, op, wait=None):
                return emit(
                    lambda: nc.vector.tensor_tensor(out=out, in0=a, in1=b, op=op),
                    wait=wait,
                )

            def loo_chain(src_h, src3, out_h, out3, wb_h, op, first_wait):
                soff = src3.offset
                pstep, gstep = src3.ap[0], src3.ap[1]
                ooff = out3.offset
                opp, opg = out3.ap[0], out3.ap[1]

                def sv(off, apdims):
                    return bass.AP(src_h, soff + off, [pstep, gstep] + apdims)

                if d == 2:
                    emit(
                        lambda: nc.vector.tensor_copy(out3, sv(1, [[-1, 2]])),
                        wait=first_wait,
                    )
                    return
                if d == 4:
                    t0 = tt(wb_h[:, :, 0, :], sv(0, [[2, 2]]), sv(1, [[2, 2]]), op,
                            wait=first_wait)
                    wb4 = wb_h[:, :, :, :]
                    mp_swap_b = bass.AP(wb_h, wb4.offset + 1, [wb4.ap[0], wb4.ap[1], [-1, 2], [0, 2]])
                    tt(bass.AP(out_h, ooff, [opp, opg, [2, 2], [1, 2]]),
                       sv(1, [[2, 2], [-1, 2]]), mp_swap_b, op, wait=t0)
                    return
                if d == 6:
                    wb4 = wb_h[:, :, :, :]
                    wboff = wb4.offset
                    wv = lambda off, apdims: bass.AP(wb_h, wboff + off, [wb4.ap[0], wb4.ap[1]] + apdims)
                    t0 = tt(wv(0, [[1, 3]]), sv(0, [[2, 3]]), sv(1, [[2, 3]]), op,
                            wait=first_wait)
                    tt(wv(3, [[1, 2]]), wv(1, [[-1, 2]]), wv(2, [[0, 2]]), op, wait=t0)
                    t2 = tt(wv(5, [[1, 1]]), wv(0, [[1, 1]]), wv(1, [[1, 1]]), op, wait=t0)
                    tt(bass.AP(out_h, ooff, [opp, opg, [2, 3], [1, 2]]),
                       sv(1, [[2, 3], [-1, 2]]), wv(3, [[1, 3], [0, 2]]), op, wait=t2)
                    return

                # generic: fused prefix/suffix pair chain
                def U(k):
                    return sv(k, [[d - 1 - 2 * k, 2]])

                wb4 = wb_h[:, :, :, :]
                prev_t = emit(
                    lambda: nc.vector.tensor_copy(wb_h[:, :, 0, :], U(0)),
                    wait=first_wait,
                )
                for k in range(1, d - 2):
                    prev_t = tt(wb_h[:, :, k, :], wb_h[:, :, k - 1, :], U(k), op, wait=prev_t)
                ends = bass.AP(out_h, ooff + d - 1, [opp, opg, [-(d - 1), 2]])
                tt(ends, wb_h[:, :, d - 3, :], U(d - 2), op, wait=prev_t)
                pre_view = bass.AP(wb_h, wb4.offset, [wb4.ap[0], wb4.ap[1], [2, d - 2]])
                suf_rev = bass.AP(wb_h, wb4.offset + (d - 3) * 2 + 1, [wb4.ap[0], wb4.ap[1], [-2, d - 2]])
                tt(out3[:, :, 1 : d - 1], pre_view, suf_rev, op)

            emit(lambda: nc.vector.memset(Km[:], -2147483648), wait=0)
            t_msets = emit(lambda: nc.vector.memset(Kp[:], 2147483647), wait=0)

            vector.wait_ge(s_in, 16)
            t_abs = emit(
                lambda: nc.vector.tensor_tensor(
                    out=A[:].bitcast(i32), in0=X.bitcast(i32), in1=Kp[:], op=AL.bitwise_and
                ),
                wait=t_msets,
            )
            loo_chain(XB, X3, T, T3, Wp, AL.mult, first_wait=0)
            t_prod = cnt[0]
            loo_chain(A, A3, M, M3, Wb, AL.min, first_wait=t_abs)
            t_min = cnt[0]

            t_sub = emit(lambda: nc.vector.tensor_sub(R[:], M[:], Bt), wait=t_min)
            emit(lambda: nc.vector.tensor_tensor(out=Ti[:], in0=T[:].bitcast(i32), in1=Km[:], op=AL.bitwise_and), wait=t_prod)
            emit(lambda: nc.vector.tensor_relu(out=R[:], in_=R[:]), wait=t_sub)
            vector.wait_ge(s_v, cnt[0])
            nc.vector.tensor_tensor(
                out=O[:], in0=R[:].bitcast(i32), in1=Ti[:], op=AL.bitwise_or
            ).then_inc(s_dve, 1)

    return nc


def _kernel_fast(x, bias, leader):
    Bn, E_ = x.shape
    slot_all, d, blocks, gpb = _build_slots(leader, nbatch=Bn)
    P, F = Bn * blocks, gpb * d
    key = ("fast", P, F, gpb, d)
    if key not in _NC_CACHE:
        _NC_CACHE[key] = _build_fast_nc(P, F, gpb, d)
    nc = _NC_CACHE[key]

    in_maps = []
    for c in range(NCORES):
        e = slot_all[c]                       # (blocks, gpb, d)
        valid = e >= 0
        ec = np.clip(e, 0, None)
        xs = np.where(valid[None], x[:, ec], np.float32(np.inf))
        bsv = np.where(valid, bias[0, ec], np.float32(0.0))
        bsv = np.broadcast_to(bsv[None], (Bn,) + bsv.shape)
        xb = np.concatenate(
            [xs.reshape(P, F), bsv.reshape(P, F)], axis=1
        )
        in_maps.append({"xb": np.ascontiguousarray(xb, np.float32)})

    results = _run_spmd(nc, in_maps)

    out = np.empty((Bn, E_), np.float32)
    for c in range(NCORES):
        e = slot_all[c]
        valid = e >= 0
        ys = results[c]["ys"].reshape(Bn, blocks, gpb, d)
        out[:, e[valid]] = ys[:, valid]
    return out


# ---------------------------------------------------------------------------
# Fallback 2: generic dense kernel for arbitrary (E, E) float masks.
# ---------------------------------------------------------------------------


def _build_dense_nc(Bn, E, Ec):
    """Generic dense fallback: any (E, E) float mask, mask rows sharded
    per core (Ec rows, padded with +inf). Exactly follows the reference:
        nb    = mask == 0
        w     = nb ? x : 1       -> signs = sign(prod w)  (pairwise tree)
        mins  = min(|x| + mask)  (fused add+min reduce)
        out   = signs * max(mins - bias_row, 0)
    Output layout "ys" is (Ec, Bn) (row-major per output row; host transposes).
    """
    import contextlib

    import concourse.bass as bass
    from concourse import mybir

    f32 = mybir.dt.float32
    AL = mybir.AluOpType
    AX = mybir.AxisListType

    PT = 128
    ntiles = (Ec + PT - 1) // PT
    assert Ec % ntiles == 0 and (Ec // ntiles) <= PT
    TR = Ec // ntiles  # rows per tile

    nc = bass.Bass()
    mrows = nc.declare_dram_parameter("mrows", [Ec, E], f32, isOutput=False)
    xfull = nc.declare_dram_parameter("xfull", [Bn, E], f32, isOutput=False)
    brows = nc.declare_dram_parameter("brows", [Ec, 1], f32, isOutput=False)
    ys = nc.declare_dram_parameter("ys", [Ec, Bn], f32, isOutput=True)

    with contextlib.ExitStack() as ctx:
        XB = []
        for b in range(Bn):
            XB.append(ctx.enter_context(nc.sbuf_tensor(f"XBc{b}", [TR, E], f32)))
        MT = ctx.enter_context(nc.sbuf_tensor("MT", [TR, E], f32))
        W = ctx.enter_context(nc.sbuf_tensor("W", [TR, E], f32))
        SC = ctx.enter_context(nc.sbuf_tensor("SC", [TR, E], f32))
        SC2 = ctx.enter_context(nc.sbuf_tensor("SC2", [TR, E], f32))
        BC = ctx.enter_context(nc.sbuf_tensor("BC", [TR, 1], f32))
        MI = ctx.enter_context(nc.sbuf_tensor("MI", [TR, 1], f32))
        SG = ctx.enter_context(nc.sbuf_tensor("SG", [TR, 1], f32))
        PR = ctx.enter_context(nc.sbuf_tensor("PR", [TR, 1], f32))
        OT = ctx.enter_context(nc.sbuf_tensor("OT", [TR, Bn], f32))

        s_bc = ctx.enter_context(nc.semaphore("s_bc"))
        s_m = ctx.enter_context(nc.semaphore("s_m"))
        s_v = ctx.enter_context(nc.semaphore("s_v"))
        s_t = ctx.enter_context(nc.semaphore("s_t"))
        s_out = ctx.enter_context(nc.semaphore("s_out"))
        block = ctx.enter_context(nc.Block())

        @block.sync
        def _(sync):
            # broadcast each batch row of x across TR partitions (stride-0 AP)
            for b in range(Bn):
                src = bass.AP(xfull, b * E, [[0, TR], [1, E]])
                sync.dma_start(out=XB[b][:], in_=src).then_inc(s_bc, 16)
            for t in range(ntiles):
                if t:
                    sync.wait_ge(s_t, t)
                    sync.dma_start(
                        out=ys[(t - 1) * TR : t * TR, :], in_=OT[:]
                    ).then_inc(s_out, 16)
                sync.dma_start(out=MT[:], in_=mrows[t * TR : (t + 1) * TR, :]).then_inc(s_m, 16)
                sync.dma_start(out=BC[:], in_=brows[t * TR : (t + 1) * TR, :]).then_inc(s_m, 16)
            sync.wait_ge(s_t, ntiles)
            sync.dma_start(
                out=ys[(ntiles - 1) * TR : ntiles * TR, :], in_=OT[:]
            ).then_inc(s_out, 16)
            sync.wait_ge(s_out, 16 * ntiles)

        @block.vector
        def _(vector):
            cnt = [0]
            waited = [0]

            def emit(fn, wait=None):
                if wait is None:
                    wait = cnt[0]
                if wait > waited[0]:
                    vector.wait_ge(s_v, wait)
                    waited[0] = wait
                fn().then_inc(s_v, 1)
                cnt[0] += 1
                return cnt[0]

            vector.wait_ge(s_bc, 16 * Bn)
            for t in range(ntiles):
                vector.wait_ge(s_m, 32 * (t + 1))
                if t:
                    vector.wait_ge(s_out, 16 * t)
                emit(lambda: nc.vector.tensor_single_scalar(out=W[:], in_=MT[:], scalar=0.0, op=AL.is_equal))
                for b in range(Bn):
                    emit(lambda b=b: nc.vector.tensor_scalar_mul(SC2[:], XB[b][:], -1.0))
                    emit(lambda b=b: nc.vector.tensor_max(SC2[:], SC2[:], XB[b][:]))
                    emit(lambda: nc.vector.tensor_add(SC[:], MT[:], SC2[:]))
                    emit(lambda: nc.vector.tensor_reduce(
                        out=MI[:], in_=SC[:], axis=AX.X, op=AL.min))
                    emit(lambda b=b: nc.vector.tensor_scalar_add(SC[:], XB[b][:], -1.0))
                    emit(lambda: nc.vector.tensor_mul(SC[:], W[:], SC[:]))
                    emit(lambda: nc.vector.tensor_scalar_add(SC[:], SC[:], 1.0))
                    n = E
                    cur, other = SC, SC2
                    while n > 1:
                        h = n // 2
                        ce = cur[:, 0 : 2 * h].rearrange("p (h two) -> p h two", two=2)
                        emit(lambda ce=ce, other=other, h=h: nc.vector.tensor_tensor(
                            out=other[:, 0:h], in0=ce[:, :, 0:1], in1=ce[:, :, 1:2], op=AL.mult))
                        if n % 2:
                            emit(lambda cur=cur, other=other, n=n: nc.vector.tensor_mul(
                                other[:, 0:1], other[:, 0:1], cur[:, n - 1 : n]))
                        cur, other = other, cur
                        n = h
                    emit(lambda cur=cur: nc.vector.tensor_single_scalar(out=SG[:], in_=cur[:, 0:1], scalar=0.0, op=AL.is_gt))
                    emit(lambda cur=cur: nc.vector.tensor_single_scalar(out=PR[:], in_=cur[:, 0:1], scalar=0.0, op=AL.is_lt))
                    emit(lambda: nc.vector.tensor_sub(SG[:], SG[:], PR[:]))
                    emit(lambda: nc.vector.tensor_scalar(
                        out=MI[:], in0=MI[:], scalar1=BC[:], scalar2=0.0,
                        op0=AL.subtract, op1=AL.max))
                    emit(lambda b=b: nc.vector.tensor_mul(OT[:, b : b + 1], SG[:], MI[:]))
                vector.wait_ge(s_v, cnt[0])
                nc.vector.engine_nop().then_inc(s_t, 1)

    return nc


def _kernel_dense(x, bias, inf_mask):
    Bn, E = x.shape
    m = np.ascontiguousarray(np.asarray(inf_mask), np.float32)
    Ec = -(-E // NCORES)
    PT = 128
    ntiles = -(-Ec // PT)
    Ec = ntiles * PT if Ec > PT else Ec
    key = ("dense", Bn, E, Ec)
    if key not in _NC_CACHE:
        _NC_CACHE[key] = _build_dense_nc(Bn, E, Ec)
    nc = _NC_CACHE[key]

    in_maps = []
    for c in range(NCORES):
        lo = c * Ec
        rows = np.full((Ec, E), np.float32(np.inf), np.float32)
        bcol = np.zeros((Ec, 1), np.float32)
        hi = min(lo + Ec, E)
        if hi > lo:
            rows[: hi - lo] = m[lo:hi]
            bcol[: hi - lo, 0] = bias[0, lo:hi]
        in_maps.append(
            {
                "mrows": rows,
                "xfull": np.ascontiguousarray(x, np.float32),
                "brows": bcol,
            }
        )

    results = _run_spmd(nc, in_maps)

    out = np.empty((Bn, E), np.float32)
    for c in range(NCORES):
        lo = c * Ec
        hi = min(lo + Ec, E)
        if hi > lo:
            out[:, lo:hi] = results[c]["ys"][: hi - lo].T
    return out


# revision 5
# speedup vs baseline: 7.3382x; 1.0135x over previous
"""Trainium2 Bass kernel for nn_EvenLayer (LDPC min-sum check-node update).

Reference semantics (B=8 batches, E=3600 edges):
    neighbor = inf_mask == 0            # (E, E)
    signs    = sign(prod(where(neighbor, x, 1), axis=-1))
    mins     = min(|x| + inf_mask, axis=-1)
    out      = signs * max(mins - bias, 0)

The mask encodes "shares a check node, excluding self" — an equivalence
relation minus the diagonal. The host verifies that structure at runtime
(values only {0, +inf}, empty diagonal, rows = leader-equality classes);
on success each edge-group (check node, size d=6) is packed into slots,
sharded over the 8 cores, and a small SPMD kernel computes per slot:
    loo_min  = leave-one-out min of |x| over the group  (tournament tree)
    loo_sign = sign bit of the leave-one-out product    (tournament tree)
    out      = relu(loo_min - bias) with loo_sign OR'd into the sign bit
which is bit-exact vs the reference.

Data movement uses the GPSIMD (Pool-engine) indirect-DMA path:
    in : dma_gather   (DRAM row i -> SBUF partition i, identity indices)
    out: dma_scatter_add (SBUF partition i -> DRAM row i; the runtime
         pre-zeros ExternalOutput buffers, so the add is a plain write)
with the index vector generated on-device via iota. All compute runs on
the Pool engine; the whole program is single-engine with no heavyweight
HWDGE legs on the critical path.

If mask verification fails, a generic dense kernel computes the masked
reductions directly from the mask data (including the reference's
product-underflow semantics for signs).
"""

import numpy as np

B, E, NCORES = 8, 3600, 8
RPAD = 128          # gather/scatter partition count (fixed by the ISA)
IDXC = RPAD // 16   # idx columns (idxs wrapped in 16 partitions)
SRC_ROWS = 256      # DRAM rows; idx tile garbage partitions reach 127+16*7=239

_NC_CACHE = {}
TRACE = False
LAST_RESULT = None  # BassKernelResults of the last run (for test harness)


def _analyze(inf_mask):
    """Return leader labels if the mask is exactly an equivalence relation
    minus the diagonal with values {0, +inf}; else None."""
    m = np.asarray(inf_mask)
    if m.ndim != 2 or m.shape[0] != m.shape[1]:
        return None
    if not np.all((m == 0) | np.isposinf(m)):
        return None
    nb = m == 0
    if nb.diagonal().any():
        return None
    n = m.shape[0]
    idx = np.arange(n)
    first = np.argmax(nb, axis=1)
    has = nb.any(axis=1)
    leader = np.where(has, np.minimum(idx, first), idx)
    eq = leader[:, None] == leader[None, :]
    np.fill_diagonal(eq, False)
    if not np.array_equal(nb, eq):
        return None
    return leader


def _build_slots(leader, nbatch=B):
    """Pack groups into (NCORES, blocks, gpb, d) slot->edge index array (-1 pad)."""
    max_blocks = max(128 // nbatch, 1)
    order = np.argsort(leader, kind="stable")
    lead_sorted = leader[order]
    uniq, counts = np.unique(lead_sorted, return_counts=True)
    G = len(uniq)
    d = max(int(counts.max()), 2)
    G8 = ((G + NCORES - 1) // NCORES) * NCORES
    slot_edge = np.full((G8, d), -1, dtype=np.int64)
    col = np.concatenate([np.arange(c) for c in counts])
    row = np.repeat(np.arange(G), counts)
    slot_edge[row, col] = order
    Gc = G8 // NCORES
    gpb = (Gc + max_blocks - 1) // max_blocks   # groups per partition-block
    blocks = (Gc + gpb - 1) // gpb
    Gcp = blocks * gpb
    slot_all = slot_edge.reshape(NCORES, Gc, d)
    if Gcp != Gc:
        pad = np.full((NCORES, Gcp - Gc, d), -1, dtype=np.int64)
        slot_all = np.concatenate([slot_all, pad], axis=1)
    return slot_all.reshape(NCORES, blocks, gpb, d), d, blocks, gpb


def _build_gather_nc(gpb, d, ew):
    """Single-engine (Pool/GPSIMD) kernel: indirect-DMA in, float-only
    compute, indirect-DMA out. Built with Bacc so GPSIMD library reloads
    are inserted and lowered automatically.

    Walrus constraints honored: Pool tensor-tensor supports only
    add/subtract/mult (f32); scalar-form min/max/is_ge are legal; no int
    alu/bitwise ops. Hence:
      sign:  S = 2*(x >= 0) - 1            (exact +/-1; +1 for +inf pads)
      abs:   A = x * S
      T:     loo-product of S (tournament) (exact +/-1)
      M:     loo-min of A (tournament; min(a,b) = b + min(a-b, 0))
      out:   max(M - bias, 0) * T

    The idx tile must hold IDX[p, c] = (p % 16) + 16*c REPLICATED in every
    16-partition stripe (the gather/scatter ucode cores read their own
    stripe; CoreSim reads stripe 0). p % 16 is built in f32 via the
    1.5*2^23 round-to-nearest trick, then copy-cast to int16.

    DRAM "xb"/"ys" are (SRC_ROWS, ew); row r < rows packs
    [x slots | bias slots | zero pad]; ys gets [out slots] per row via
    scatter-add (the runtime zero-fills ExternalOutput buffers, so add ==
    write). elem_step=ew keeps the scatter row stride 256B-aligned while
    writing only F elements per row.
    """
    import contextlib

    import concourse.bass as bass
    from concourse.bacc import Bacc
    from concourse import mybir

    f32 = mybir.dt.float32
    i16 = mybir.dt.int16
    AL = mybir.AluOpType
    F = gpb * d

    nc = Bacc(None, target_bir_lowering=False)
    xb = nc.declare_dram_parameter("xb", [SRC_ROWS, ew], f32, isOutput=False)
    ys = nc.declare_dram_parameter("ys", [SRC_ROWS, ew], f32, isOutput=True)

    with contextlib.ExitStack() as ctx:
        IDX = ctx.enter_context(nc.sbuf_tensor("IDX", [RPAD, IDXC], i16))
        PF = ctx.enter_context(nc.sbuf_tensor("PF", [RPAD, IDXC], f32))
        CF = ctx.enter_context(nc.sbuf_tensor("CF", [RPAD, IDXC], f32))
        QF = ctx.enter_context(nc.sbuf_tensor("QF", [RPAD, IDXC], f32))
        XB = ctx.enter_context(nc.sbuf_tensor("XB", [RPAD, ew], f32))
        S = ctx.enter_context(nc.sbuf_tensor("S", [RPAD, F], f32))
        A = ctx.enter_context(nc.sbuf_tensor("A", [RPAD, F], f32))
        T = ctx.enter_context(nc.sbuf_tensor("T", [RPAD, F], f32))
        M = ctx.enter_context(nc.sbuf_tensor("M", [RPAD, F], f32))
        Wp = ctx.enter_context(nc.sbuf_tensor("Wp", [RPAD, gpb, max(d - 2, 1), 2], f32))
        Wb = ctx.enter_context(nc.sbuf_tensor("Wb", [RPAD, gpb, max(d - 2, 1), 2], f32))
        SC0 = ctx.enter_context(nc.sbuf_tensor("SC0", [RPAD, gpb, 16], f32))
        SC1 = ctx.enter_context(nc.sbuf_tensor("SC1", [RPAD, gpb, 16], f32))
        SC2 = ctx.enter_context(nc.sbuf_tensor("SC2", [RPAD, gpb, 16], f32))
        SC3 = ctx.enter_context(nc.sbuf_tensor("SC3", [RPAD, gpb, 16], f32))
        R = ctx.enter_context(nc.sbuf_tensor("R", [RPAD, F], f32))
        O = ctx.enter_context(nc.sbuf_tensor("O", [RPAD, F], f32))

        s_g = ctx.enter_context(nc.semaphore("s_g"))
        s_o = ctx.enter_context(nc.semaphore("s_o"))
        s_v = ctx.enter_context(nc.semaphore("s_v"))

        X = XB[:, 0:F]
        Bt = XB[:, F : 2 * F]

        gp = nc.gpsimd
        g = gp

        cnt = [0]
        waited = [0]

        def emit(fn, wait=None):
            if wait is None:
                wait = cnt[0]          # default: wait for all prior ops
            if wait > waited[0]:
                g.wait_ge(s_v, wait)
                waited[0] = wait
            fn().then_inc(s_v, 1)
            cnt[0] += 1
            return cnt[0]

        def tt(out, a, b, op, wait=None):
            return emit(lambda: gp.tensor_tensor(out=out, in0=a, in1=b, op=op),
                        wait=wait)

        # ---- replicated idx tile (see docstring) ----
        t_p = emit(lambda: gp.iota(PF[:], pattern=[[0, IDXC]], base=0,
                                   channel_multiplier=1,
                                   allow_small_or_imprecise_dtypes=True), wait=0)
        t_c = emit(lambda: gp.iota(CF[:], pattern=[[16, IDXC]], base=0,
                                   channel_multiplier=0,
                                   allow_small_or_imprecise_dtypes=True), wait=0)
        # QF = 16*floor(p/16) via the magic-add rounding trick: at
        # 1.5*2^27 the f32 spacing is 16, so adding the magic rounds
        # (p - 7.5) to the nearest multiple of 16 = 16*floor(p/16)
        # (|p - 7.5 - 16k| <= 7.5 < 8, no ties). The -7.5 must be applied
        # at small scale first (it is below the f32 ulp at 2^27).
        t_q0 = emit(lambda: gp.tensor_scalar(
            out=QF[:], in0=PF[:], scalar1=1.0, scalar2=7.5,
            op0=AL.mult, op1=AL.subtract), wait=t_p)
        t_q = emit(lambda: gp.tensor_scalar(
            out=QF[:], in0=QF[:], scalar1=201326592.0, scalar2=201326592.0,
            op0=AL.add, op1=AL.subtract), wait=t_q0)
        t_pm = emit(lambda: gp.tensor_sub(PF[:], PF[:], QF[:]), wait=t_q)
        t_ix = tt(PF[:], PF[:], CF[:], AL.add, wait=max(t_pm, t_c))
        t_setup = emit(lambda: gp.tensor_copy(IDX[:], PF[:]), wait=t_ix)

        g.wait_ge(s_v, t_setup)
        waited[0] = t_setup
        XB3 = bass.AP(XB, XB[:].offset, [XB[:].ap[0], [ew, 1], [1, ew]])
        gp.dma_gather(
            out_ap=XB3, in_ap=xb[:, :], idxs_ap=IDX[:],
            num_idxs=RPAD, num_idxs_reg=RPAD, elem_size=ew,
        ).then_inc(s_g, 16)
        g.wait_ge(s_g, 16)

        S3 = S[:].rearrange("p (g d) -> p g d", d=d)
        A3 = A[:].rearrange("p (g d) -> p g d", d=d)
        M3 = M[:].rearrange("p (g d) -> p g d", d=d)
        T3 = T[:].rearrange("p (g d) -> p g d", d=d)

        def views(src_h, src3):
            soff = src3.offset
            pstep, gstep = src3.ap[0], src3.ap[1]

            def sv(off, apdims):
                return bass.AP(src_h, soff + off, [pstep, gstep] + apdims)

            return sv

        def wviews(wb_h):
            wb4 = wb_h[:, :, :, :]
            wboff = wb4.offset

            def wv(off, apdims):
                return bass.AP(wb_h, wboff + off, [wb4.ap[0], wb4.ap[1]] + apdims)

            return wv

        def prod_tree(src_h, src3, out_h, out3, wb_h, first_wait):
            """d==6 group sign product broadcast to slots: T[s] = P6 * S[s].
            S is exactly +/-1, so P6 * S[s] = product of the other five."""
            sv = views(src_h, src3)
            wv = wviews(wb_h)
            ooff = out3.offset
            opp, opg = out3.ap[0], out3.ap[1]
            op = AL.mult
            # mp[k] = S[2k]*S[2k+1] -> wb 0..2; q = mp0*mp1 -> wb 3; P6 -> wb 4
            t0 = tt(wv(0, [[1, 3]]), sv(0, [[2, 3]]), sv(1, [[2, 3]]), op,
                    wait=first_wait)
            t1 = tt(wv(3, [[1, 1]]), wv(0, [[1, 1]]), wv(1, [[1, 1]]), op, wait=t0)
            t2 = tt(wv(4, [[1, 1]]), wv(3, [[1, 1]]), wv(2, [[1, 1]]), op, wait=t1)
            tt(bass.AP(out_h, ooff, [opp, opg, [1, 6]]),
               sv(0, [[1, 6]]), wv(4, [[0, 6]]), op, wait=t2)

        def min_pair(out, a, b, scr, wait):
            """out = min(a, b), bit-exact via 0/1 masks:
            d = a-b; g = [d >= 0]; h = [d < 0]; out = a*h + b*g.
            Each product multiplies by exactly 0.0 or 1.0 and one addend is
            zero, so the selected value passes through unrounded (needed:
            outputs near zero are graded at ~1e-6 absolute scale, so the
            rounding of cheaper min decompositions fails the rel-err gate)."""
            d, gm, hm, p = scr
            t0 = emit(lambda: gp.tensor_tensor(out=d, in0=a, in1=b,
                                               op=AL.subtract), wait=wait)
            t1 = emit(lambda: gp.tensor_single_scalar(out=gm, in_=d, scalar=0.0,
                                                      op=AL.is_ge), wait=t0)
            t2 = emit(lambda: gp.tensor_single_scalar(out=hm, in_=d, scalar=0.0,
                                                      op=AL.is_lt), wait=t0)
            t3 = tt(p, a, hm, AL.mult, wait=t2)
            t4 = tt(d, b, gm, AL.mult, wait=max(t1, t3))
            return tt(out, p, d, AL.add, wait=t4)

        def min_tree(src_h, src3, out_h, out3, wb_h, scr_hs, first_wait):
            """d==6 leave-one-out min tournament (exact pairwise mins)."""
            sv = views(src_h, src3)
            wv = wviews(wb_h)

            def w3views(h):
                w3 = h[:, :, :]
                w3off = w3.offset

                def wv3(off, apdims):
                    return bass.AP(h, w3off + off, [w3.ap[0], w3.ap[1]] + apdims)

                return wv3

            svs = [w3views(h) for h in scr_hs]
            ooff = out3.offset
            opp, opg = out3.ap[0], out3.ap[1]

            def scr(off, dims):
                return tuple(v(off, dims) for v in svs)

            t0 = min_pair(wv(0, [[1, 3]]), sv(0, [[2, 3]]), sv(1, [[2, 3]]),
                          scr(0, [[1, 3]]), wait=first_wait)
            min_pair(wv(3, [[1, 2]]), wv(1, [[-1, 2]]), wv(2, [[0, 2]]),
                     scr(3, [[1, 2]]), wait=t0)
            t2 = min_pair(wv(5, [[1, 1]]), wv(0, [[1, 1]]), wv(1, [[1, 1]]),
                          scr(5, [[1, 1]]), wait=t0)
            min_pair(bass.AP(out_h, ooff, [opp, opg, [2, 3], [1, 2]]),
                     sv(1, [[2, 3], [-1, 2]]), wv(3, [[1, 3], [0, 2]]),
                     tuple(v(8, [[1, 6]]) for v in svs),
                     wait=t2)

        t_s01 = emit(lambda: gp.tensor_single_scalar(
            out=S[:], in_=X, scalar=0.0, op=AL.is_ge))
        t_sgn = emit(lambda: gp.tensor_scalar(
            out=S[:], in0=S[:], scalar1=2.0, scalar2=1.0,
            op0=AL.mult, op1=AL.subtract), wait=t_s01)
        t_abs = tt(A[:], X, S[:], AL.mult, wait=t_sgn)
        prod_tree(S, S3, T, T3, Wp, first_wait=t_sgn)
        t_prod = cnt[0]
        min_tree(A, A3, M, M3, Wb, [SC0, SC1, SC2, SC3], first_wait=t_abs)
        t_min = cnt[0]
        t_sub = emit(lambda: gp.tensor_sub(R[:], M[:], Bt), wait=t_min)
        t_relu = emit(lambda: gp.tensor_scalar_max(R[:], R[:], 0.0), wait=t_sub)
        t_o = tt(O[:], R[:], T[:], AL.mult, wait=max(t_relu, t_prod))

        g.wait_ge(s_v, t_o)
        O3 = bass.AP(O, O[:].offset, [O[:].ap[0], [F, 1], [1, F]])
        ys_ap = bass.AP(ys, 0, [[ew, SRC_ROWS], [1, F]])
        gp.dma_scatter_add(
            out_ap=ys_ap, in_ap=O3, idxs_ap=IDX[:],
            num_idxs=RPAD, num_idxs_reg=RPAD, elem_size=F, elem_step=ew,
        ).then_inc(s_o, 16)
        g.wait_ge(s_o, 16)

    nc.finalize()
    return nc


def _prepare_gather(x, bias, leader):
    """Build (nc, in_maps, unpack) for the gather-kernel path, or None if the
    problem shape doesn't fit it."""
    Bn, E_ = x.shape
    slot_all, d, blocks, gpb = _build_slots(leader, nbatch=Bn)
    rows = Bn * blocks
    F = gpb * d
    ew = ((2 * F + 63) // 64) * 64      # gather/scatter element: 256B aligned
    if d != 6 or rows > RPAD or ew > SRC_ROWS:
        return None

    key = ("gather", gpb, d, ew)
    if key not in _NC_CACHE:
        _NC_CACHE[key] = _build_gather_nc(gpb, d, ew)
    nc = _NC_CACHE[key]

    in_maps = []
    for c in range(NCORES):
        e = slot_all[c]                       # (blocks, gpb, d)
        valid = e >= 0
        ec = np.clip(e, 0, None)
        xs = np.where(valid[None], x[:, ec], np.float32(1e30))
        bsv = np.where(valid, bias[0, ec], np.float32(0.0))
        bsv = np.broadcast_to(bsv[None], (Bn,) + bsv.shape)
        src = np.zeros((SRC_ROWS, ew), np.float32)
        src[:rows, 0:F] = xs.reshape(rows, F)
        src[:rows, F : 2 * F] = bsv.reshape(rows, F)
        in_maps.append({"xb": src})

    def unpack(results):
        out = np.empty((Bn, E_), np.float32)
        for c in range(NCORES):
            e = slot_all[c]
            valid = e >= 0
            ysr = results[c]["ys"][:rows, 0:F].reshape(Bn, blocks, gpb, d)
            out[:, e[valid]] = ysr[:, valid]
        return out

    return nc, in_maps, unpack


def _run_spmd(nc, in_maps):
    global LAST_RESULT
    from concourse.bass_utils import run_bass_kernel_spmd

    res = run_bass_kernel_spmd(
        nc, in_maps, core_ids=list(range(NCORES)), trace=TRACE
    )
    LAST_RESULT = res
    return res.results


def kernel(inputs, bias, inf_mask):
    x = np.ascontiguousarray(np.asarray(inputs), np.float32)
    bias = np.ascontiguousarray(np.asarray(bias), np.float32)
    inf_mask = np.asarray(inf_mask)

    leader = _analyze(inf_mask)
    if leader is not None:
        prep = _prepare_gather(x, bias, leader)
        if prep is not None:
            nc, in_maps, unpack = prep
            return unpack(_run_spmd(nc, in_maps))
        return _kernel_fast(x, bias, leader)
    return _kernel_dense(x, bias, inf_mask)


# ---------------------------------------------------------------------------
# Fallback 1: HWDGE-DMA + DVE kernel for equivalence-class masks whose shape
# doesn't fit the gather kernel (d != 6 or too many rows).
# ---------------------------------------------------------------------------


def _build_fast_nc(P, F, gpb, d):
    """Raw-bass kernel: one DMA in, DVE compute, one DMA out.

    Input "xb" packs [x_slots | bias_slots] as (P, 2F); output "ys" is (P, F).
    """
    import contextlib

    import concourse.bass as bass
    from concourse import mybir

    f32 = mybir.dt.float32
    i32 = mybir.dt.int32
    AL = mybir.AluOpType

    nc = bass.Bass()
    xb = nc.declare_dram_parameter("xb", [P, 2 * F], f32, isOutput=False)
    ys = nc.declare_dram_parameter("ys", [P, F], f32, isOutput=True)

    with contextlib.ExitStack() as ctx:
        XB = ctx.enter_context(nc.sbuf_tensor("XB", [P, 2 * F], f32))
        A = ctx.enter_context(nc.sbuf_tensor("A", [P, F], f32))
        T = ctx.enter_context(nc.sbuf_tensor("T", [P, F], f32))
        Ti = ctx.enter_context(nc.sbuf_tensor("Ti", [P, F], i32))
        Km = ctx.enter_context(nc.sbuf_tensor("Km", [P, F], i32))
        Kp = ctx.enter_context(nc.sbuf_tensor("Kp", [P, F], i32))
        M = ctx.enter_context(nc.sbuf_tensor("M", [P, F], f32))
        Wb = ctx.enter_context(nc.sbuf_tensor("Wb", [P, gpb, max(d - 2, 1), 2], f32))
        Wp = ctx.enter_context(nc.sbuf_tensor("Wp", [P, gpb, max(d - 2, 1), 2], f32))
        R = ctx.enter_context(nc.sbuf_tensor("R", [P, F], f32))
        O = ctx.enter_context(nc.sbuf_tensor("O", [P, F], i32))

        s_in = ctx.enter_context(nc.semaphore("s_in"))
        s_dve = ctx.enter_context(nc.semaphore("s_dve"))
        s_out = ctx.enter_context(nc.semaphore("s_out"))
        s_v = ctx.enter_context(nc.semaphore("s_v"))
        block = ctx.enter_context(nc.Block())

        X = XB[:, 0:F]
        Bt = XB[:, F : 2 * F]

        @block.sync
        def _(sync):
            sync.dma_start(out=XB[:], in_=xb[:]).then_inc(s_in, 16)
            sync.wait_ge(s_dve, 1)
            sync.dma_start(out=ys[:], in_=O[:].bitcast(f32)).then_inc(s_out, 16)
            sync.wait_ge(s_out, 16)

        @block.vector
        def _(vector):
            X3 = X.rearrange("p (g d) -> p g d", d=d)
            A3 = A[:].rearrange("p (g d) -> p g d", d=d)
            M3 = M[:].rearrange("p (g d) -> p g d", d=d)
            T3 = T[:].rearrange("p (g d) -> p g d", d=d)

            cnt = [0]
            waited = [0]

            def emit(fn, wait=None):
                if wait is None:
                    wait = cnt[0]
                if wait > waited[0]:
                    vector.wait_ge(s_v, wait)
                    waited[0] = wait
                fn().then_inc(s_v, 1)
                cnt[0] += 1
                return cnt[0]

            def tt(out, a, b, op, wait=None):
                return emit(
                    lambda: nc.vector.tensor_tensor(out=out, in0=a, in1=b, op=op),
                    wait=wait,
                )

            def loo_chain(src_h, src3, out_h, out3, wb_h, op, first_wait):
                soff = src3.offset
                pstep, gstep = src3.ap[0], src3.ap[1]
                ooff = out3.offset
                opp, opg = out3.ap[0], out3.ap[1]

                def sv(off, apdims):
                    return bass.AP(src_h, soff + off, [pstep, gstep] + apdims)

                if d == 2:
                    emit(
                        lambda: nc.vector.tensor_copy(out3, sv(1, [[-1, 2]])),
                        wait=first_wait,
                    )
                    return
                if d == 4:
                    t0 = tt(wb_h[:, :, 0, :], sv(0, [[2, 2]]), sv(1, [[2, 2]]), op,
                            wait=first_wait)
                    wb4 = wb_h[:, :, :, :]
                    mp_swap_b = bass.AP(wb_h, wb4.offset + 1, [wb4.ap[0], wb4.ap[1], [-1, 2], [0, 2]])
                    tt(bass.AP(out_h, ooff, [opp, opg, [2, 2], [1, 2]]),
                       sv(1, [[2, 2], [-1, 2]]), mp_swap_b, op, wait=t0)
                    return
                if d == 6:
                    wb4 = wb_h[:, :, :, :]
                    wboff = wb4.offset
                    wv = lambda off, apdims: bass.AP(wb_h, wboff + off, [wb4.ap[0], wb4.ap[1]] + apdims)
                    t0 = tt(wv(0, [[1, 3]]), sv(0, [[2, 3]]), sv(1, [[2, 3]]), op,
                            wait=first_wait)
                    tt(wv(3, [[1, 2]]), wv(1, [[-1, 2]]), wv(2, [[0, 2]]), op, wait=t0)
                    t2 = tt(wv(5, [[1, 1]]), wv(0, [[1, 1]]), wv(1, [[1, 1]]), op, wait=t0)
                    tt(bass.AP(out_h, ooff, [opp, opg, [2, 3], [1, 2]]),
                       sv(1, [[2, 3], [-1, 2]]), wv(3, [[1, 3], [0, 2]]), op, wait=t2)
                    return

                # generic: fused prefix/suffix pair chain
                def U(k):
                    return sv(k, [[d - 1 - 2 * k, 2]])

                wb4 = wb_h[:, :, :, :]
                prev_t = emit(
                    lambda: nc.vector.tensor_copy(wb_h[:, :, 0, :], U(0)),
                    wait=first_wait,
                )
                for k in range(1, d - 2):
                    prev_t = tt(wb_h[:, :, k, :], wb_h[:, :, k - 1, :], U(k), op, wait=prev_t)
                ends = bass.AP(out_h, ooff + d - 1, [opp, opg, [-(d - 1), 2]])
                tt(ends, wb_h[:, :, d - 3, :], U(d - 2), op, wait=prev_t)
                pre_view = bass.AP(wb_h, wb4.offset, [wb4.ap[0], wb4.ap[1], [2, d - 2]])
                suf_rev = bass.AP(wb_h, wb4.offset + (d - 3) * 2 + 1, [wb4.ap[0], wb4.ap[1], [-2, d - 2]])
                tt(out3[:, :, 1 : d - 1], pre_view, suf_rev, op)

            emit(lambda: nc.vector.memset(Km[:], -2147483648), wait=0)
            t_msets = emit(lambda: nc.vector.memset(Kp[:], 2147483647), wait=0)

            vector.wait_ge(s_in, 16)
            t_abs = emit(
                lambda: nc.vector.tensor_tensor(
                    out=A[:].bitcast(i32), in0=X.bitcast(i32), in1=Kp[:], op=AL.bitwise_and
                ),
                wait=t_msets,
            )
            loo_chain(XB, X3, T, T3, Wp, AL.mult, first_wait=0)
            t_prod = cnt[0]
            loo_chain(A, A3, M, M3, Wb, AL.min, first_wait=t_abs)
            t_min = cnt[0]

            t_sub = emit(lambda: nc.vector.tensor_sub(R[:], M[:], Bt), wait=t_min)
            emit(lambda: nc.vector.tensor_tensor(out=Ti[:], in0=T[:].bitcast(i32), in1=Km[:], op=AL.bitwise_and), wait=t_prod)
            emit(lambda: nc.vector.tensor_relu(out=R[:], in_=R[:]), wait=t_sub)
            vector.wait_ge(s_v, cnt[0])
            nc.vector.tensor_tensor(
                out=O[:], in0=R[:].bitcast(i32), in1=Ti[:], op=AL.bitwise_or
            ).then_inc(s_dve, 1)

    return nc


def _kernel_fast(x, bias, leader):
    Bn, E_ = x.shape
    slot_all, d, blocks, gpb = _build_slots(leader, nbatch=Bn)
    P, F = Bn * blocks, gpb * d
    key = ("fast", P, F, gpb, d)
    if key not in _NC_CACHE:
        _NC_CACHE[key] = _build_fast_nc(P, F, gpb, d)
    nc = _NC_CACHE[key]

    in_maps = []
    for c in range(NCORES):
        e = slot_all[c]                       # (blocks, gpb, d)
        valid = e >= 0
        ec = np.clip(e, 0, None)
        xs = np.where(valid[None], x[:, ec], np.float32(np.inf))
        bsv = np.where(valid, bias[0, ec], np.float32(0.0))
        bsv = np.broadcast_to(bsv[None], (Bn,) + bsv.shape)
        xb = np.concatenate(
            [xs.reshape(P, F), bsv.reshape(P, F)], axis=1
        )
        in_maps.append({"xb": np.ascontiguousarray(xb, np.float32)})

    results = _run_spmd(nc, in_maps)

    out = np.empty((Bn, E_), np.float32)
    for c in range(NCORES):
        e = slot_all[c]
        valid = e >= 0
        ys = results[c]["ys"].reshape(Bn, blocks, gpb, d)
        out[:, e[valid]] = ys[:, valid]
    return out


# ---------------------------------------------------------------------------
# Fallback 2: generic dense kernel for arbitrary (E, E) float masks.
# ---------------------------------------------------------------------------


def _build_dense_nc(Bn, E, Ec):
    """Generic dense fallback: any (E, E) float mask, mask rows sharded
    per core (Ec rows, padded with +inf). Exactly follows the reference:
        nb    = mask == 0
        w     = nb ? x : 1       -> signs = sign(prod w)  (pairwise tree)
        mins  = min(|x| + mask)  (fused add+min reduce)
        out   = signs * max(mins - bias_row, 0)
    Output layout "ys" is (Ec, Bn) (row-major per output row; host transposes).
    """
    import contextlib

    import concourse.bass as bass
    from concourse import mybir

    f32 = mybir.dt.float32
    AL = mybir.AluOpType
    AX = mybir.AxisListType

    PT = 128
    ntiles = (Ec + PT - 1) // PT
    assert Ec % ntiles == 0 and (Ec // ntiles) <= PT
    TR = Ec // ntiles  # rows per tile

    nc = bass.Bass()
    mrows = nc.declare_dram_parameter("mrows", [Ec, E], f32, isOutput=False)
    xfull = nc.declare_dram_parameter("xfull", [Bn, E], f32, isOutput=False)
    brows = nc.declare_dram_parameter("brows", [Ec, 1], f32, isOutput=False)
    ys = nc.declare_dram_parameter("ys", [Ec, Bn], f32, isOutput=True)

    with contextlib.ExitStack() as ctx:
        XB = []
        for b in range(Bn):
            XB.append(ctx.enter_context(nc.sbuf_tensor(f"XBc{b}", [TR, E], f32)))
        MT = ctx.enter_context(nc.sbuf_tensor("MT", [TR, E], f32))
        W = ctx.enter_context(nc.sbuf_tensor("W", [TR, E], f32))
        SC = ctx.enter_context(nc.sbuf_tensor("SC", [TR, E], f32))
        SC2 = ctx.enter_context(nc.sbuf_tensor("SC2", [TR, E], f32))
        BC = ctx.enter_context(nc.sbuf_tensor("BC", [TR, 1], f32))
        MI = ctx.enter_context(nc.sbuf_tensor("MI", [TR, 1], f32))
        SG = ctx.enter_context(nc.sbuf_tensor("SG", [TR, 1], f32))
        PR = ctx.enter_context(nc.sbuf_tensor("PR", [TR, 1], f32))
        OT = ctx.enter_context(nc.sbuf_tensor("OT", [TR, Bn], f32))

        s_bc = ctx.enter_context(nc.semaphore("s_bc"))
        s_m = ctx.enter_context(nc.semaphore("s_m"))
        s_v = ctx.enter_context(nc.semaphore("s_v"))
        s_t = ctx.enter_context(nc.semaphore("s_t"))
        s_out = ctx.enter_context(nc.semaphore("s_out"))
        block = ctx.enter_context(nc.Block())

        @block.sync
        def _(sync):
            # broadcast each batch row of x across TR partitions (stride-0 AP)
            for b in range(Bn):
                src = bass.AP(xfull, b * E, [[0, TR], [1, E]])
                sync.dma_start(out=XB[b][:], in_=src).then_inc(s_bc, 16)
            for t in range(ntiles):
                if t:
                    sync.wait_ge(s_t, t)
                    sync.dma_start(
                        out=ys[(t - 1) * TR : t * TR, :], in_=OT[:]
                    ).then_inc(s_out, 16)
                sync.dma_start(out=MT[:], in_=mrows[t * TR : (t + 1) * TR, :]).then_inc(s_m, 16)
                sync.dma_start(out=BC[:], in_=brows[t * TR : (t + 1) * TR, :]).then_inc(s_m, 16)
            sync.wait_ge(s_t, ntiles)
            sync.dma_start(
                out=ys[(ntiles - 1) * TR : ntiles * TR, :], in_=OT[:]
            ).then_inc(s_out, 16)
            sync.wait_ge(s_out, 16 * ntiles)

        @block.vector
        def _(vector):
            cnt = [0]
            waited = [0]

            def emit(fn, wait=None):
                if wait is None:
                    wait = cnt[0]
                if wait > waited[0]:
                    vector.wait_ge(s_v, wait)
                    waited[0] = wait
                fn().then_inc(s_v, 1)
                cnt[0] += 1
                return cnt[0]

            vector.wait_ge(s_bc, 16 * Bn)
            for t in range(ntiles):
                vector.wait_ge(s_m, 32 * (t + 1))
                if t:
                    vector.wait_ge(s_out, 16 * t)
                emit(lambda: nc.vector.tensor_single_scalar(out=W[:], in_=MT[:], scalar=0.0, op=AL.is_equal))
                for b in range(Bn):
                    emit(lambda b=b: nc.vector.tensor_scalar_mul(SC2[:], XB[b][:], -1.0))
                    emit(lambda b=b: nc.vector.tensor_max(SC2[:], SC2[:], XB[b][:]))
                    emit(lambda: nc.vector.tensor_add(SC[:], MT[:], SC2[:]))
                    emit(lambda: nc.vector.tensor_reduce(
                        out=MI[:], in_=SC[:], axis=AX.X, op=AL.min))
                    emit(lambda b=b: nc.vector.tensor_scalar_add(SC[:], XB[b][:], -1.0))
                    emit(lambda: nc.vector.tensor_mul(SC[:], W[:], SC[:]))
                    emit(lambda: nc.vector.tensor_scalar_add(SC[:], SC[:], 1.0))
                    n = E
                    cur, other = SC, SC2
                    while n > 1:
                        h = n // 2
                        ce = cur[:, 0 : 2 * h].rearrange("p (h two) -> p h two", two=2)
                        emit(lambda ce=ce, other=other, h=h: nc.vector.tensor_tensor(
                            out=other[:, 0:h], in0=ce[:, :, 0:1], in1=ce[:, :, 1:2], op=AL.mult))
                        if n % 2:
                            emit(lambda cur=cur, other=other, n=n: nc.vector.tensor_mul(
                                other[:, 0:1], other[:, 0:1], cur[:, n - 1 : n]))
                        cur, other = other, cur
                        n = h
                    emit(lambda cur=cur: nc.vector.tensor_single_scalar(out=SG[:], in_=cur[:, 0:1], scalar=0.0, op=AL.is_gt))
                    emit(lambda cur=cur: nc.vector.tensor_single_scalar(out=PR[:], in_=cur[:, 0:1], scalar=0.0, op=AL.is_lt))
                    emit(lambda: nc.vector.tensor_sub(SG[:], SG[:], PR[:]))
                    emit(lambda: nc.vector.tensor_scalar(
                        out=MI[:], in0=MI[:], scalar1=BC[:], scalar2=0.0,
                        op0=AL.subtract, op1=AL.max))
                    emit(lambda b=b: nc.vector.tensor_mul(OT[:, b : b + 1], SG[:], MI[:]))
                vector.wait_ge(s_v, cnt[0])
                nc.vector.engine_nop().then_inc(s_t, 1)

    return nc


def _kernel_dense(x, bias, inf_mask):
    Bn, E = x.shape
    m = np.ascontiguousarray(np.asarray(inf_mask), np.float32)
    Ec = -(-E // NCORES)
    PT = 128
    ntiles = -(-Ec // PT)
    Ec = ntiles * PT if Ec > PT else Ec
    key = ("dense", Bn, E, Ec)
    if key not in _NC_CACHE:
        _NC_CACHE[key] = _build_dense_nc(Bn, E, Ec)
    nc = _NC_CACHE[key]

    in_maps = []
    for c in range(NCORES):
        lo = c * Ec
        rows = np.full((Ec, E), np.float32(np.inf), np.float32)
        bcol = np.zeros((Ec, 1), np.float32)
        hi = min(lo + Ec, E)
        if hi > lo:
            rows[: hi - lo] = m[lo:hi]
            bcol[: hi - lo, 0] = bias[0, lo:hi]
        in_maps.append(
            {
                "mrows": rows,
                "xfull": np.ascontiguousarray(x, np.float32),
                "brows": bcol,
            }
        )

    results = _run_spmd(nc, in_maps)

    out = np.empty((Bn, E), np.float32)
    for c in range(NCORES):
        lo = c * Ec
        hi = min(lo + Ec, E)
        if hi > lo:
            out[:, lo:hi] = results[c]["ys"][: hi - lo].T
    return out


# revision 6
# speedup vs baseline: 8.3631x; 1.1397x over previous
"""Trainium2 Bass kernel for nn_EvenLayer (LDPC min-sum check-node update).

Reference semantics (B=8 batches, E=3600 edges):
    neighbor = inf_mask == 0            # (E, E)
    signs    = sign(prod(where(neighbor, x, 1), axis=-1))
    mins     = min(|x| + inf_mask, axis=-1)
    out      = signs * max(mins - bias, 0)

The mask encodes "shares a check node, excluding self" — an equivalence
relation minus the diagonal. The host verifies that structure at runtime
(values only {0, +inf}, empty diagonal, rows = leader-equality classes);
on success each edge-group (check node, size d=6) is packed into slots,
sharded over the 8 cores, and a small SPMD kernel computes per slot:
    loo_min  = leave-one-out min of |x| over the group  (tournament tree)
    loo_sign = sign bit of the leave-one-out product    (tournament tree)
    out      = relu(loo_min - bias) with loo_sign OR'd into the sign bit
which is bit-exact vs the reference.

Data movement uses the GPSIMD (Pool-engine) indirect-DMA path:
    in : dma_gather   (DRAM row i -> SBUF partition i, identity indices)
    out: dma_scatter_add (SBUF partition i -> DRAM row i; the runtime
         pre-zeros ExternalOutput buffers, so the add is a plain write)
with the index vector generated on-device via iota. All compute runs on
the Pool engine; the whole program is single-engine with no heavyweight
HWDGE legs on the critical path.

If mask verification fails, a generic dense kernel computes the masked
reductions directly from the mask data (including the reference's
product-underflow semantics for signs).
"""

import numpy as np

B, E, NCORES = 8, 3600, 8
RPAD = 128          # gather/scatter partition count (fixed by the ISA)
IDXC = RPAD // 16   # idx columns (idxs wrapped in 16 partitions)
SRC_ROWS = 256      # DRAM rows; idx tile garbage partitions reach 127+16*7=239

_NC_CACHE = {}
TRACE = False
LAST_RESULT = None  # BassKernelResults of the last run (for test harness)


def _analyze(inf_mask):
    """Return leader labels if the mask is exactly an equivalence relation
    minus the diagonal with values {0, +inf}; else None."""
    m = np.asarray(inf_mask)
    if m.ndim != 2 or m.shape[0] != m.shape[1]:
        return None
    if not np.all((m == 0) | np.isposinf(m)):
        return None
    nb = m == 0
    if nb.diagonal().any():
        return None
    n = m.shape[0]
    idx = np.arange(n)
    first = np.argmax(nb, axis=1)
    has = nb.any(axis=1)
    leader = np.where(has, np.minimum(idx, first), idx)
    eq = leader[:, None] == leader[None, :]
    np.fill_diagonal(eq, False)
    if not np.array_equal(nb, eq):
        return None
    return leader


def _build_slots(leader, nbatch=B):
    """Pack groups into (NCORES, blocks, gpb, d) slot->edge index array (-1 pad)."""
    max_blocks = max(128 // nbatch, 1)
    order = np.argsort(leader, kind="stable")
    lead_sorted = leader[order]
    uniq, counts = np.unique(lead_sorted, return_counts=True)
    G = len(uniq)
    d = max(int(counts.max()), 2)
    G8 = ((G + NCORES - 1) // NCORES) * NCORES
    slot_edge = np.full((G8, d), -1, dtype=np.int64)
    col = np.concatenate([np.arange(c) for c in counts])
    row = np.repeat(np.arange(G), counts)
    slot_edge[row, col] = order
    Gc = G8 // NCORES
    gpb = (Gc + max_blocks - 1) // max_blocks   # groups per partition-block
    blocks = (Gc + gpb - 1) // gpb
    Gcp = blocks * gpb
    slot_all = slot_edge.reshape(NCORES, Gc, d)
    if Gcp != Gc:
        pad = np.full((NCORES, Gcp - Gc, d), -1, dtype=np.int64)
        slot_all = np.concatenate([slot_all, pad], axis=1)
    return slot_all.reshape(NCORES, blocks, gpb, d), d, blocks, gpb


def _build_gather_nc(gpb, d, ew):
    """Single-engine (Pool/GPSIMD) kernel: indirect-DMA in, float-only
    compute, indirect-DMA out. Built with Bacc so GPSIMD library reloads
    are inserted and lowered automatically.

    Walrus constraints honored: Pool tensor-tensor supports only
    add/subtract/mult (f32); scalar-form min/max/is_ge are legal; no int
    alu/bitwise ops. Hence:
      sign:  S = 2*(x >= 0) - 1            (exact +/-1; +1 for +inf pads)
      abs:   A = x * S
      T:     loo-product of S (tournament) (exact +/-1)
      M:     loo-min of A (tournament; min(a,b) = b + min(a-b, 0))
      out:   max(M - bias, 0) * T

    The idx tile must hold IDX[p, c] = (p % 16) + 16*c REPLICATED in every
    16-partition stripe (the gather/scatter ucode cores read their own
    stripe; CoreSim reads stripe 0). p % 16 is built in f32 via the
    1.5*2^23 round-to-nearest trick, then copy-cast to int16.

    DRAM "xb"/"ys" are (SRC_ROWS, ew); row r < rows packs
    [x slots | bias slots | zero pad]; ys gets [out slots] per row via
    scatter-add (the runtime zero-fills ExternalOutput buffers, so add ==
    write). elem_step=ew keeps the scatter row stride 256B-aligned while
    writing only F elements per row.
    """
    import contextlib

    import concourse.bass as bass
    from concourse.bacc import Bacc
    from concourse import mybir

    f32 = mybir.dt.float32
    i16 = mybir.dt.int16
    AL = mybir.AluOpType
    F = gpb * d

    nc = Bacc(None, target_bir_lowering=False)
    xb = nc.declare_dram_parameter("xb", [SRC_ROWS, ew], f32, isOutput=False)
    ys = nc.declare_dram_parameter("ys", [SRC_ROWS, ew], f32, isOutput=True)

    with contextlib.ExitStack() as ctx:
        IDX = ctx.enter_context(nc.sbuf_tensor("IDX", [RPAD, IDXC], i16))
        PF = ctx.enter_context(nc.sbuf_tensor("PF", [RPAD, IDXC], f32))
        CF = ctx.enter_context(nc.sbuf_tensor("CF", [RPAD, IDXC], f32))
        QF = ctx.enter_context(nc.sbuf_tensor("QF", [RPAD, IDXC], f32))
        XB = ctx.enter_context(nc.sbuf_tensor("XB", [RPAD, ew], f32))
        S = ctx.enter_context(nc.sbuf_tensor("S", [RPAD, F], f32))
        A = ctx.enter_context(nc.sbuf_tensor("A", [RPAD, F], f32))
        T = ctx.enter_context(nc.sbuf_tensor("T", [RPAD, F], f32))
        M = ctx.enter_context(nc.sbuf_tensor("M", [RPAD, F], f32))
        Wp = ctx.enter_context(nc.sbuf_tensor("Wp", [RPAD, gpb, max(d - 2, 1), 2], f32))
        Wb = ctx.enter_context(nc.sbuf_tensor("Wb", [RPAD, gpb, max(d - 2, 1), 2], f32))
        SC0 = ctx.enter_context(nc.sbuf_tensor("SC0", [RPAD, gpb, 16], f32))
        SC1 = ctx.enter_context(nc.sbuf_tensor("SC1", [RPAD, gpb, 16], f32))
        SC2 = ctx.enter_context(nc.sbuf_tensor("SC2", [RPAD, gpb, 16], f32))
        SC3 = ctx.enter_context(nc.sbuf_tensor("SC3", [RPAD, gpb, 16], f32))
        R = ctx.enter_context(nc.sbuf_tensor("R", [RPAD, F], f32))
        O = ctx.enter_context(nc.sbuf_tensor("O", [RPAD, F], f32))

        s_g = ctx.enter_context(nc.semaphore("s_g"))
        s_o = ctx.enter_context(nc.semaphore("s_o"))
        s_v = ctx.enter_context(nc.semaphore("s_v"))

        X = XB[:, 0:F]
        Bt = XB[:, F : 2 * F]

        gp = nc.gpsimd
        g = gp

        cnt = [0]
        waited = [0]

        def emit(fn, wait=None):
            if wait is None:
                wait = cnt[0]          # default: wait for all prior ops
            if wait > waited[0]:
                g.wait_ge(s_v, wait)
                waited[0] = wait
            fn().then_inc(s_v, 1)
            cnt[0] += 1
            return cnt[0]

        def tt(out, a, b, op, wait=None):
            return emit(lambda: gp.tensor_tensor(out=out, in0=a, in1=b, op=op),
                        wait=wait)

        # ---- replicated idx tile (see docstring) ----
        t_p = emit(lambda: gp.iota(PF[:], pattern=[[0, IDXC]], base=0,
                                   channel_multiplier=1,
                                   allow_small_or_imprecise_dtypes=True), wait=0)
        t_c = emit(lambda: gp.iota(CF[:], pattern=[[16, IDXC]], base=0,
                                   channel_multiplier=0,
                                   allow_small_or_imprecise_dtypes=True), wait=0)
        # QF = 16*floor(p/16) via the magic-add rounding trick: at
        # 1.5*2^27 the f32 spacing is 16, so adding the magic rounds
        # (p - 7.5) to the nearest multiple of 16 = 16*floor(p/16)
        # (|p - 7.5 - 16k| <= 7.5 < 8, no ties). The -7.5 must be applied
        # at small scale first (it is below the f32 ulp at 2^27).
        t_q0 = emit(lambda: gp.tensor_scalar(
            out=QF[:], in0=PF[:], scalar1=1.0, scalar2=7.5,
            op0=AL.mult, op1=AL.subtract), wait=t_p)
        t_q = emit(lambda: gp.tensor_scalar(
            out=QF[:], in0=QF[:], scalar1=201326592.0, scalar2=201326592.0,
            op0=AL.add, op1=AL.subtract), wait=t_q0)
        t_pm = emit(lambda: gp.tensor_sub(PF[:], PF[:], QF[:]), wait=t_q)
        t_ix = tt(PF[:], PF[:], CF[:], AL.add, wait=max(t_pm, t_c))
        t_setup = emit(lambda: gp.tensor_copy(IDX[:], PF[:]), wait=t_ix)

        g.wait_ge(s_v, t_setup)
        waited[0] = t_setup
        XB3 = bass.AP(XB, XB[:].offset, [XB[:].ap[0], [ew, 1], [1, ew]])
        gp.dma_gather(
            out_ap=XB3, in_ap=xb[:, :], idxs_ap=IDX[:],
            num_idxs=RPAD, num_idxs_reg=RPAD, elem_size=ew,
        ).then_inc(s_g, 16)
        g.wait_ge(s_g, 16)

        S3 = S[:].rearrange("p (g d) -> p g d", d=d)
        A3 = A[:].rearrange("p (g d) -> p g d", d=d)
        M3 = M[:].rearrange("p (g d) -> p g d", d=d)
        T3 = T[:].rearrange("p (g d) -> p g d", d=d)

        def views(src_h, src3):
            soff = src3.offset
            pstep, gstep = src3.ap[0], src3.ap[1]

            def sv(off, apdims):
                return bass.AP(src_h, soff + off, [pstep, gstep] + apdims)

            return sv

        def wviews(wb_h):
            wb4 = wb_h[:, :, :, :]
            wboff = wb4.offset

            def wv(off, apdims):
                return bass.AP(wb_h, wboff + off, [wb4.ap[0], wb4.ap[1]] + apdims)

            return wv

        def prod_tree(src_h, src3, out_h, out3, wb_h, first_wait):
            """d==6 group sign product broadcast to slots: T[s] = P6 * S[s].
            S is exactly +/-1, so P6 * S[s] = product of the other five."""
            sv = views(src_h, src3)
            wv = wviews(wb_h)
            ooff = out3.offset
            opp, opg = out3.ap[0], out3.ap[1]
            op = AL.mult
            # mp[k] = S[2k]*S[2k+1] -> wb 0..2; q = mp0*mp1 -> wb 3; P6 -> wb 4
            t0 = tt(wv(0, [[1, 3]]), sv(0, [[2, 3]]), sv(1, [[2, 3]]), op,
                    wait=first_wait)
            t1 = tt(wv(3, [[1, 1]]), wv(0, [[1, 1]]), wv(1, [[1, 1]]), op, wait=t0)
            t2 = tt(wv(4, [[1, 1]]), wv(3, [[1, 1]]), wv(2, [[1, 1]]), op, wait=t1)
            tt(bass.AP(out_h, ooff, [opp, opg, [1, 6]]),
               sv(0, [[1, 6]]), wv(4, [[0, 6]]), op, wait=t2)

        def min_pair(out, a, b, scr, wait):
            """out = min(a, b), bit-exact via 0/1 masks:
            d = a-b; g = [d >= 0]; h = [d < 0]; out = a*h + b*g.
            Each product multiplies by exactly 0.0 or 1.0 and one addend is
            zero, so the selected value passes through unrounded (needed:
            outputs near zero are graded at ~1e-6 absolute scale, so the
            rounding of cheaper min decompositions fails the rel-err gate)."""
            d, gm, hm, p = scr
            t0 = emit(lambda: gp.tensor_tensor(out=d, in0=a, in1=b,
                                               op=AL.subtract), wait=wait)
            t1 = emit(lambda: gp.tensor_single_scalar(out=gm, in_=d, scalar=0.0,
                                                      op=AL.is_ge), wait=t0)
            t2 = emit(lambda: gp.tensor_single_scalar(out=hm, in_=d, scalar=0.0,
                                                      op=AL.is_lt), wait=t0)
            t3 = tt(p, a, hm, AL.mult, wait=t2)
            t4 = tt(d, b, gm, AL.mult, wait=max(t1, t3))
            return tt(out, p, d, AL.add, wait=t4)

        def min_tree(src_h, src3, out_h, out3, wb_h, scr_hs, first_wait):
            """d==6 leave-one-out min tournament (exact pairwise mins)."""
            sv = views(src_h, src3)
            wv = wviews(wb_h)

            def w3views(h):
                w3 = h[:, :, :]
                w3off = w3.offset

                def wv3(off, apdims):
                    return bass.AP(h, w3off + off, [w3.ap[0], w3.ap[1]] + apdims)

                return wv3

            svs = [w3views(h) for h in scr_hs]
            ooff = out3.offset
            opp, opg = out3.ap[0], out3.ap[1]

            def scr(off, dims):
                return tuple(v(off, dims) for v in svs)

            t0 = min_pair(wv(0, [[1, 3]]), sv(0, [[2, 3]]), sv(1, [[2, 3]]),
                          scr(0, [[1, 3]]), wait=first_wait)
            min_pair(wv(3, [[1, 2]]), wv(1, [[-1, 2]]), wv(2, [[0, 2]]),
                     scr(3, [[1, 2]]), wait=t0)
            t2 = min_pair(wv(5, [[1, 1]]), wv(0, [[1, 1]]), wv(1, [[1, 1]]),
                          scr(5, [[1, 1]]), wait=t0)
            min_pair(bass.AP(out_h, ooff, [opp, opg, [2, 3], [1, 2]]),
                     sv(1, [[2, 3], [-1, 2]]), wv(3, [[1, 3], [0, 2]]),
                     tuple(v(8, [[1, 6]]) for v in svs),
                     wait=t2)

        t_s01 = emit(lambda: gp.tensor_single_scalar(
            out=S[:], in_=X, scalar=0.0, op=AL.is_ge))
        t_sgn = emit(lambda: gp.tensor_scalar(
            out=S[:], in0=S[:], scalar1=2.0, scalar2=1.0,
            op0=AL.mult, op1=AL.subtract), wait=t_s01)
        t_abs = tt(A[:], X, S[:], AL.mult, wait=t_sgn)
        prod_tree(S, S3, T, T3, Wp, first_wait=t_sgn)
        t_prod = cnt[0]
        min_tree(A, A3, M, M3, Wb, [SC0, SC1, SC2, SC3], first_wait=t_abs)
        t_min = cnt[0]
        t_sub = emit(lambda: gp.tensor_sub(R[:], M[:], Bt), wait=t_min)
        t_relu = emit(lambda: gp.tensor_scalar_max(R[:], R[:], 0.0), wait=t_sub)
        t_o = tt(O[:], R[:], T[:], AL.mult, wait=max(t_relu, t_prod))

        g.wait_ge(s_v, t_o)
        O3 = bass.AP(O, O[:].offset, [O[:].ap[0], [F, 1], [1, F]])
        ys_ap = bass.AP(ys, 0, [[ew, SRC_ROWS], [1, F]])
        gp.dma_scatter_add(
            out_ap=ys_ap, in_ap=O3, idxs_ap=IDX[:],
            num_idxs=RPAD, num_idxs_reg=RPAD, elem_size=F, elem_step=ew,
        ).then_inc(s_o, 16)
        g.wait_ge(s_o, 16)

    # Re-lead the entry barrier: stock all_engine_barrier makes Pool the
    # releaser (wait gather>=4 from the other engines, then release), which
    # puts ~100ns of barrier latency at the head of the single working
    # engine's stream. Reassign the wait+release pair to SP (which idles
    # anyway): SP's own Drain already contributes one of the 4 gather incs,
    # so the count stays correct, and Pool's stream starts immediately.
    # Pool's bare Drain moves to the end of its stream where its latency
    # overlaps the final DMA-semaphore tail.
    entry = nc.main_func.blocks[0]
    insts = entry.instructions
    pool_eng = nc.gpsimd.engine
    sp_eng = nc.sync.engine
    trio_start = None
    for i, ins in enumerate(insts):
        if ins.engine == pool_eng and type(ins).__name__ == "InstDrain":
            trio_start = i
            break
    assert trio_start is not None
    trio = [insts[trio_start]]
    j = trio_start + 1
    while j < len(insts) and len(trio) < 3:
        if insts[j].engine == pool_eng:
            assert type(insts[j]).__name__ == "InstEventSemaphore", insts[j]
            trio.append(insts[j])
        j += 1
    assert len(trio) == 3
    drain, wait_gather, release = trio
    sp_drain_idx = None
    for i, ins in enumerate(insts):
        if ins.engine == sp_eng and type(ins).__name__ == "InstDrain":
            sp_drain_idx = i
            break
    assert sp_drain_idx is not None and sp_drain_idx < trio_start
    wait_gather.engine = sp_eng
    release.engine = sp_eng
    insts.remove(wait_gather)
    insts.remove(release)
    insts.remove(drain)
    insts.insert(sp_drain_idx + 1, wait_gather)
    insts.insert(sp_drain_idx + 2, release)
    insts.append(drain)

    nc.finalize()
    return nc


def _prepare_gather(x, bias, leader):
    """Build (nc, in_maps, unpack) for the gather-kernel path, or None if the
    problem shape doesn't fit it."""
    Bn, E_ = x.shape
    slot_all, d, blocks, gpb = _build_slots(leader, nbatch=Bn)
    rows = Bn * blocks
    F = gpb * d
    ew = ((2 * F + 63) // 64) * 64      # gather/scatter element: 256B aligned
    if d != 6 or rows > RPAD or ew > SRC_ROWS:
        return None

    key = ("gather", gpb, d, ew)
    if key not in _NC_CACHE:
        _NC_CACHE[key] = _build_gather_nc(gpb, d, ew)
    nc = _NC_CACHE[key]

    in_maps = []
    for c in range(NCORES):
        e = slot_all[c]                       # (blocks, gpb, d)
        valid = e >= 0
        ec = np.clip(e, 0, None)
        xs = np.where(valid[None], x[:, ec], np.float32(1e30))
        bsv = np.where(valid, bias[0, ec], np.float32(0.0))
        bsv = np.broadcast_to(bsv[None], (Bn,) + bsv.shape)
        src = np.zeros((SRC_ROWS, ew), np.float32)
        src[:rows, 0:F] = xs.reshape(rows, F)
        src[:rows, F : 2 * F] = bsv.reshape(rows, F)
        in_maps.append({"xb": src})

    def unpack(results):
        out = np.empty((Bn, E_), np.float32)
        for c in range(NCORES):
            e = slot_all[c]
            valid = e >= 0
            ysr = results[c]["ys"][:rows, 0:F].reshape(Bn, blocks, gpb, d)
            out[:, e[valid]] = ysr[:, valid]
        return out

    return nc, in_maps, unpack


def _run_spmd(nc, in_maps):
    global LAST_RESULT
    from concourse.bass_utils import run_bass_kernel_spmd

    res = run_bass_kernel_spmd(
        nc, in_maps, core_ids=list(range(NCORES)), trace=TRACE
    )
    LAST_RESULT = res
    return res.results


def kernel(inputs, bias, inf_mask):
    x = np.ascontiguousarray(np.asarray(inputs), np.float32)
    bias = np.ascontiguousarray(np.asarray(bias), np.float32)
    inf_mask = np.asarray(inf_mask)

    leader = _analyze(inf_mask)
    if leader is not None:
        prep = _prepare_gather(x, bias, leader)
        if prep is not None:
            nc, in_maps, unpack = prep
            return unpack(_run_spmd(nc, in_maps))
        return _kernel_fast(x, bias, leader)
    return _kernel_dense(x, bias, inf_mask)


# ---------------------------------------------------------------------------
# Fallback 1: HWDGE-DMA + DVE kernel for equivalence-class masks whose shape
# doesn't fit the gather kernel (d != 6 or too many rows).
# ---------------------------------------------------------------------------


def _build_fast_nc(P, F, gpb, d):
    """Raw-bass kernel: one DMA in, DVE compute, one DMA out.

    Input "xb" packs [x_slots | bias_slots] as (P, 2F); output "ys" is (P, F).
    """
    import contextlib

    import concourse.bass as bass
    from concourse import mybir

    f32 = mybir.dt.float32
    i32 = mybir.dt.int32
    AL = mybir.AluOpType

    nc = bass.Bass()
    xb = nc.declare_dram_parameter("xb", [P, 2 * F], f32, isOutput=False)
    ys = nc.declare_dram_parameter("ys", [P, F], f32, isOutput=True)

    with contextlib.ExitStack() as ctx:
        XB = ctx.enter_context(nc.sbuf_tensor("XB", [P, 2 * F], f32))
        A = ctx.enter_context(nc.sbuf_tensor("A", [P, F], f32))
        T = ctx.enter_context(nc.sbuf_tensor("T", [P, F], f32))
        Ti = ctx.enter_context(nc.sbuf_tensor("Ti", [P, F], i32))
        Km = ctx.enter_context(nc.sbuf_tensor("Km", [P, F], i32))
        Kp = ctx.enter_context(nc.sbuf_tensor("Kp", [P, F], i32))
        M = ctx.enter_context(nc.sbuf_tensor("M", [P, F], f32))
        Wb = ctx.enter_context(nc.sbuf_tensor("Wb", [P, gpb, max(d - 2, 1), 2], f32))
        Wp = ctx.enter_context(nc.sbuf_tensor("Wp", [P, gpb, max(d - 2, 1), 2], f32))
        R = ctx.enter_context(nc.sbuf_tensor("R", [P, F], f32))
        O = ctx.enter_context(nc.sbuf_tensor("O", [P, F], i32))

        s_in = ctx.enter_context(nc.semaphore("s_in"))
        s_dve = ctx.enter_context(nc.semaphore("s_dve"))
        s_out = ctx.enter_context(nc.semaphore("s_out"))
        s_v = ctx.enter_context(nc.semaphore("s_v"))
        block = ctx.enter_context(nc.Block())

        X = XB[:, 0:F]
        Bt = XB[:, F : 2 * F]

        @block.sync
        def _(sync):
            sync.dma_start(out=XB[:], in_=xb[:]).then_inc(s_in, 16)
            sync.wait_ge(s_dve, 1)
            sync.dma_start(out=ys[:], in_=O[:].bitcast(f32)).then_inc(s_out, 16)
            sync.wait_ge(s_out, 16)

        @block.vector
        def _(vector):
            X3 = X.rearrange("p (g d) -> p g d", d=d)
            A3 = A[:].rearrange("p (g d) -> p g d", d=d)
            M3 = M[:].rearrange("p (g d) -> p g d", d=d)
            T3 = T[:].rearrange("p (g d) -> p g d", d=d)

            cnt = [0]
            waited = [0]

            def emit(fn, wait=None):
                if wait is None:
                    wait = cnt[0]
                if wait > waited[0]:
                    vector.wait_ge(s_v, wait)
                    waited[0] = wait
                fn().then_inc(s_v, 1)
                cnt[0] += 1
                return cnt[0]

            def tt(out, a, b, op, wait=None):
                return emit(
                    lambda: nc.vector.tensor_tensor(out=out, in0=a, in1=b, op=op),
                    wait=wait,
                )

            def loo_chain(src_h, src3, out_h, out3, wb_h, op, first_wait):
                soff = src3.offset
                pstep, gstep = src3.ap[0], src3.ap[1]
                ooff = out3.offset
                opp, opg = out3.ap[0], out3.ap[1]

                def sv(off, apdims):
                    return bass.AP(src_h, soff + off, [pstep, gstep] + apdims)

                if d == 2:
                    emit(
                        lambda: nc.vector.tensor_copy(out3, sv(1, [[-1, 2]])),
                        wait=first_wait,
                    )
                    return
                if d == 4:
                    t0 = tt(wb_h[:, :, 0, :], sv(0, [[2, 2]]), sv(1, [[2, 2]]), op,
                            wait=first_wait)
                    wb4 = wb_h[:, :, :, :]
                    mp_swap_b = bass.AP(wb_h, wb4.offset + 1, [wb4.ap[0], wb4.ap[1], [-1, 2], [0, 2]])
                    tt(bass.AP(out_h, ooff, [opp, opg, [2, 2], [1, 2]]),
                       sv(1, [[2, 2], [-1, 2]]), mp_swap_b, op, wait=t0)
                    return
                if d == 6:
                    wb4 = wb_h[:, :, :, :]
                    wboff = wb4.offset
                    wv = lambda off, apdims: bass.AP(wb_h, wboff + off, [wb4.ap[0], wb4.ap[1]] + apdims)
                    t0 = tt(wv(0, [[1, 3]]), sv(0, [[2, 3]]), sv(1, [[2, 3]]), op,
                            wait=first_wait)
                    tt(wv(3, [[1, 2]]), wv(1, [[-1, 2]]), wv(2, [[0, 2]]), op, wait=t0)
                    t2 = tt(wv(5, [[1, 1]]), wv(0, [[1, 1]]), wv(1, [[1, 1]]), op, wait=t0)
                    tt(bass.AP(out_h, ooff, [opp, opg, [2, 3], [1, 2]]),
                       sv(1, [[2, 3], [-1, 2]]), wv(3, [[1, 3], [0, 2]]), op, wait=t2)
                    return

                # generic: fused prefix/suffix pair chain
                def U(k):
                    return sv(k, [[d - 1 - 2 * k, 2]])

                wb4 = wb_h[:, :, :, :]
                prev_t = emit(
                    lambda: nc.vector.tensor_copy(wb_h[:, :, 0, :], U(0)),
                    wait=first_wait,
                )
                for k in range(1, d - 2):
                    prev_t = tt(wb_h[:, :, k, :], wb_h[:, :, k - 1, :], U(k), op, wait=prev_t)
                ends = bass.AP(out_h, ooff + d - 1, [opp, opg, [-(d - 1), 2]])
                tt(ends, wb_h[:, :, d - 3, :], U(d - 2), op, wait=prev_t)
                pre_view = bass.AP(wb_h, wb4.offset, [wb4.ap[0], wb4.ap[1], [2, d - 2]])
                suf_rev = bass.AP(wb_h, wb4.offset + (d - 3) * 2 + 1, [wb4.ap[0], wb4.ap[1], [-2, d - 2]])
                tt(out3[:, :, 1 : d - 1], pre_view, suf_rev, op)

            emit(lambda: nc.vector.memset(Km[:], -2147483648), wait=0)
            t_msets = emit(lambda: nc.vector.memset(Kp[:], 2147483647), wait=0)

            vector.wait_ge(s_in, 16)
            t_abs = emit(
                lambda: nc.vector.tensor_tensor(
                    out=A[:].bitcast(i32), in0=X.bitcast(i32), in1=Kp[:], op=AL.bitwise_and
                ),
                wait=t_msets,
            )
            loo_chain(XB, X3, T, T3, Wp, AL.mult, first_wait=0)
            t_prod = cnt[0]
            loo_chain(A, A3, M, M3, Wb, AL.min, first_wait=t_abs)
            t_min = cnt[0]

            t_sub = emit(lambda: nc.vector.tensor_sub(R[:], M[:], Bt), wait=t_min)
            emit(lambda: nc.vector.tensor_tensor(out=Ti[:], in0=T[:].bitcast(i32), in1=Km[:], op=AL.bitwise_and), wait=t_prod)
            emit(lambda: nc.vector.tensor_relu(out=R[:], in_=R[:]), wait=t_sub)
            vector.wait_ge(s_v, cnt[0])
            nc.vector.tensor_tensor(
                out=O[:], in0=R[:].bitcast(i32), in1=Ti[:], op=AL.bitwise_or
            ).then_inc(s_dve, 1)

    return nc


def _kernel_fast(x, bias, leader):
    Bn, E_ = x.shape
    slot_all, d, blocks, gpb = _build_slots(leader, nbatch=Bn)
    P, F = Bn * blocks, gpb * d
    key = ("fast", P, F, gpb, d)
    if key not in _NC_CACHE:
        _NC_CACHE[key] = _build_fast_nc(P, F, gpb, d)
    nc = _NC_CACHE[key]

    in_maps = []
    for c in range(NCORES):
        e = slot_all[c]                       # (blocks, gpb, d)
        valid = e >= 0
        ec = np.clip(e, 0, None)
        xs = np.where(valid[None], x[:, ec], np.float32(np.inf))
        bsv = np.where(valid, bias[0, ec], np.float32(0.0))
        bsv = np.broadcast_to(bsv[None], (Bn,) + bsv.shape)
        xb = np.concatenate(
            [xs.reshape(P, F), bsv.reshape(P, F)], axis=1
        )
        in_maps.append({"xb": np.ascontiguousarray(xb, np.float32)})

    results = _run_spmd(nc, in_maps)

    out = np.empty((Bn, E_), np.float32)
    for c in range(NCORES):
        e = slot_all[c]
        valid = e >= 0
        ys = results[c]["ys"].reshape(Bn, blocks, gpb, d)
        out[:, e[valid]] = ys[:, valid]
    return out


# ---------------------------------------------------------------------------
# Fallback 2: generic dense kernel for arbitrary (E, E) float masks.
# ---------------------------------------------------------------------------


def _build_dense_nc(Bn, E, Ec):
    """Generic dense fallback: any (E, E) float mask, mask rows sharded
    per core (Ec rows, padded with +inf). Exactly follows the reference:
        nb    = mask == 0
        w     = nb ? x : 1       -> signs = sign(prod w)  (pairwise tree)
        mins  = min(|x| + mask)  (fused add+min reduce)
        out   = signs * max(mins - bias_row, 0)
    Output layout "ys" is (Ec, Bn) (row-major per output row; host transposes).
    """
    import contextlib

    import concourse.bass as bass
    from concourse import mybir

    f32 = mybir.dt.float32
    AL = mybir.AluOpType
    AX = mybir.AxisListType

    PT = 128
    ntiles = (Ec + PT - 1) // PT
    assert Ec % ntiles == 0 and (Ec // ntiles) <= PT
    TR = Ec // ntiles  # rows per tile

    nc = bass.Bass()
    mrows = nc.declare_dram_parameter("mrows", [Ec, E], f32, isOutput=False)
    xfull = nc.declare_dram_parameter("xfull", [Bn, E], f32, isOutput=False)
    brows = nc.declare_dram_parameter("brows", [Ec, 1], f32, isOutput=False)
    ys = nc.declare_dram_parameter("ys", [Ec, Bn], f32, isOutput=True)

    with contextlib.ExitStack() as ctx:
        XB = []
        for b in range(Bn):
            XB.append(ctx.enter_context(nc.sbuf_tensor(f"XBc{b}", [TR, E], f32)))
        MT = ctx.enter_context(nc.sbuf_tensor("MT", [TR, E], f32))
        W = ctx.enter_context(nc.sbuf_tensor("W", [TR, E], f32))
        SC = ctx.enter_context(nc.sbuf_tensor("SC", [TR, E], f32))
        SC2 = ctx.enter_context(nc.sbuf_tensor("SC2", [TR, E], f32))
        BC = ctx.enter_context(nc.sbuf_tensor("BC", [TR, 1], f32))
        MI = ctx.enter_context(nc.sbuf_tensor("MI", [TR, 1], f32))
        SG = ctx.enter_context(nc.sbuf_tensor("SG", [TR, 1], f32))
        PR = ctx.enter_context(nc.sbuf_tensor("PR", [TR, 1], f32))
        OT = ctx.enter_context(nc.sbuf_tensor("OT", [TR, Bn], f32))

        s_bc = ctx.enter_context(nc.semaphore("s_bc"))
        s_m = ctx.enter_context(nc.semaphore("s_m"))
        s_v = ctx.enter_context(nc.semaphore("s_v"))
        s_t = ctx.enter_context(nc.semaphore("s_t"))
        s_out = ctx.enter_context(nc.semaphore("s_out"))
        block = ctx.enter_context(nc.Block())

        @block.sync
        def _(sync):
            # broadcast each batch row of x across TR partitions (stride-0 AP)
            for b in range(Bn):
                src = bass.AP(xfull, b * E, [[0, TR], [1, E]])
                sync.dma_start(out=XB[b][:], in_=src).then_inc(s_bc, 16)
            for t in range(ntiles):
                if t:
                    sync.wait_ge(s_t, t)
                    sync.dma_start(
                        out=ys[(t - 1) * TR : t * TR, :], in_=OT[:]
                    ).then_inc(s_out, 16)
                sync.dma_start(out=MT[:], in_=mrows[t * TR : (t + 1) * TR, :]).then_inc(s_m, 16)
                sync.dma_start(out=BC[:], in_=brows[t * TR : (t + 1) * TR, :]).then_inc(s_m, 16)
            sync.wait_ge(s_t, ntiles)
            sync.dma_start(
                out=ys[(ntiles - 1) * TR : ntiles * TR, :], in_=OT[:]
            ).then_inc(s_out, 16)
            sync.wait_ge(s_out, 16 * ntiles)

        @block.vector
        def _(vector):
            cnt = [0]
            waited = [0]

            def emit(fn, wait=None):
                if wait is None:
                    wait = cnt[0]
                if wait > waited[0]:
                    vector.wait_ge(s_v, wait)
                    waited[0] = wait
                fn().then_inc(s_v, 1)
                cnt[0] += 1
                return cnt[0]

            vector.wait_ge(s_bc, 16 * Bn)
            for t in range(ntiles):
                vector.wait_ge(s_m, 32 * (t + 1))
                if t:
                    vector.wait_ge(s_out, 16 * t)
                emit(lambda: nc.vector.tensor_single_scalar(out=W[:], in_=MT[:], scalar=0.0, op=AL.is_equal))
                for b in range(Bn):
                    emit(lambda b=b: nc.vector.tensor_scalar_mul(SC2[:], XB[b][:], -1.0))
                    emit(lambda b=b: nc.vector.tensor_max(SC2[:], SC2[:], XB[b][:]))
                    emit(lambda: nc.vector.tensor_add(SC[:], MT[:], SC2[:]))
                    emit(lambda: nc.vector.tensor_reduce(
                        out=MI[:], in_=SC[:], axis=AX.X, op=AL.min))
                    emit(lambda b=b: nc.vector.tensor_scalar_add(SC[:], XB[b][:], -1.0))
                    emit(lambda: nc.vector.tensor_mul(SC[:], W[:], SC[:]))
                    emit(lambda: nc.vector.tensor_scalar_add(SC[:], SC[:], 1.0))
                    n = E
                    cur, other = SC, SC2
                    while n > 1:
                        h = n // 2
                        ce = cur[:, 0 : 2 * h].rearrange("p (h two) -> p h two", two=2)
                        emit(lambda ce=ce, other=other, h=h: nc.vector.tensor_tensor(
                            out=other[:, 0:h], in0=ce[:, :, 0:1], in1=ce[:, :, 1:2], op=AL.mult))
                        if n % 2:
                            emit(lambda cur=cur, other=other, n=n: nc.vector.tensor_mul(
                                other[:, 0:1], other[:, 0:1], cur[:, n - 1 : n]))
                        cur, other = other, cur
                        n = h
                    emit(lambda cur=cur: nc.vector.tensor_single_scalar(out=SG[:], in_=cur[:, 0:1], scalar=0.0, op=AL.is_gt))
                    emit(lambda cur=cur: nc.vector.tensor_single_scalar(out=PR[:], in_=cur[:, 0:1], scalar=0.0, op=AL.is_lt))
                    emit(lambda: nc.vector.tensor_sub(SG[:], SG[:], PR[:]))
                    emit(lambda: nc.vector.tensor_scalar(
                        out=MI[:], in0=MI[:], scalar1=BC[:], scalar2=0.0,
                        op0=AL.subtract, op1=AL.max))
                    emit(lambda b=b: nc.vector.tensor_mul(OT[:, b : b + 1], SG[:], MI[:]))
                vector.wait_ge(s_v, cnt[0])
                nc.vector.engine_nop().then_inc(s_t, 1)

    return nc


def _kernel_dense(x, bias, inf_mask):
    Bn, E = x.shape
    m = np.ascontiguousarray(np.asarray(inf_mask), np.float32)
    Ec = -(-E // NCORES)
    PT = 128
    ntiles = -(-Ec // PT)
    Ec = ntiles * PT if Ec > PT else Ec
    key = ("dense", Bn, E, Ec)
    if key not in _NC_CACHE:
        _NC_CACHE[key] = _build_dense_nc(Bn, E, Ec)
    nc = _NC_CACHE[key]

    in_maps = []
    for c in range(NCORES):
        lo = c * Ec
        rows = np.full((Ec, E), np.float32(np.inf), np.float32)
        bcol = np.zeros((Ec, 1), np.float32)
        hi = min(lo + Ec, E)
        if hi > lo:
            rows[: hi - lo] = m[lo:hi]
            bcol[: hi - lo, 0] = bias[0, lo:hi]
        in_maps.append(
            {
                "mrows": rows,
                "xfull": np.ascontiguousarray(x, np.float32),
                "brows": bcol,
            }
        )

    results = _run_spmd(nc, in_maps)

    out = np.empty((Bn, E), np.float32)
    for c in range(NCORES):
        lo = c * Ec
        hi = min(lo + Ec, E)
        if hi > lo:
            out[:, lo:hi] = results[c]["ys"][: hi - lo].T
    return out
